# revision 13
# baseline (speedup 1.0000x reference)
"""Trainium2 Bass kernel for a causal multi-head attention layer.

Model: b=2, s=2048, d_model=1024, 16 heads, head_dim=64, pad-index 0.
Sharding over 8 NeuronCores: each core owns 2 heads (128 of the 1024
attention dims) for both batches (head/tensor parallel).  After attention,
AllToAlls redistribute the per-head outputs so each core holds all 1024
attention dims for 1/8 of the sequence positions, where it runs the output
projection locally.  Output rows per core: 512 (4 chunks of 128).

Schedule: the exp of the attention scores (ACT engine, ~88us for both
batches) is the critical chain, and the PE stream blocks on the 2-deep
score-PSUM ring whenever it runs more than 2 score entries ahead of the
ACT engine.  So the emitter rate-matches: score entries are woven one at
a time between ~1us micro-units of projection / PV / output-projection
work.  A2As run as 4 collectives (one per half-batch) fired as soon as
each half is normalized; a dummy collective at the top absorbs the
collective-stream warmup and core-launch skew.
"""

import threading

import numpy as np

B, S, D = 2, 2048, 1024
H, HD = 16, 64
NCORES = 8
LD = D // NCORES          # 128 local attention dims (2 heads)
R = B * S                 # 4096 flattened rows
RC = R // NCORES          # 512 output rows per core
NKT = S // 128            # 16 key tiles per batch
NCH = D // 128            # 8 contraction chunks of d_model
NST = S // 512            # 4 query stripes per batch

_cache = {}
_lock = threading.Lock()


def _stripe_layout():
    """Per stripe c: list of (kt, width, q_start, offset-in-block), block len."""
    layout = []
    for c in range(NST):
        entries = []
        off = 0
        for kt in range(4 * c + 4):
            qs = max(512 * c, kt * 128)
            w = 512 * (c + 1) - qs
            entries.append((kt, w, qs, off))
            off += w
        layout.append((entries, off))
    return layout


def _build_nc():
    import concourse.mybir as mybir
    import concourse.tile as tile
    from concourse import bacc
    from contextlib import ExitStack

    f32 = mybir.dt.float32
    bf16 = mybir.dt.bfloat16
    i32 = mybir.dt.int32
    AF = mybir.ActivationFunctionType
    ALU = mybir.AluOpType

    nc = bacc.Bacc(None, target_bir_lowering=False, num_devices=NCORES)

    xT = nc.declare_dram_parameter("xT", [D, R], bf16, isOutput=False)
    wqT = nc.declare_dram_parameter("wqT", [D, LD], bf16, isOutput=False)
    wkT = nc.declare_dram_parameter("wkT", [D, LD], bf16, isOutput=False)
    wvT = nc.declare_dram_parameter("wvT", [D, LD], bf16, isOutput=False)
    woT = nc.declare_dram_parameter("woT", [D, D], bf16, isOutput=False)
    bq = nc.declare_dram_parameter("bq", [LD], f32, isOutput=False)
    bk = nc.declare_dram_parameter("bk", [LD], f32, isOutput=False)
    bv = nc.declare_dram_parameter("bv", [LD], f32, isOutput=False)
    bo = nc.declare_dram_parameter("bo", [D], f32, isOutput=False)
    ids = nc.declare_dram_parameter("ids", [128, B * NKT], i32, isOutput=False)
    out = nc.declare_dram_parameter("out", [RC, D], f32, isOutput=True)

    layout = _stripe_layout()

    with ExitStack() as ctx:
        tc = ctx.enter_context(tile.TileContext(nc))
        const = ctx.enter_context(tc.tile_pool(name="const", bufs=1))
        xcp = ctx.enter_context(tc.tile_pool(name="xcp", bufs=1))
        qkp = ctx.enter_context(tc.tile_pool(name="qkp", bufs=2))
        estp = ctx.enter_context(tc.tile_pool(name="estp", bufs=1))
        stg = ctx.enter_context(tc.tile_pool(name="stg", bufs=2))
        work = ctx.enter_context(tc.tile_pool(name="work", bufs=2))
        recp = ctx.enter_context(tc.tile_pool(name="recp", bufs=1))
        ppool = ctx.enter_context(tc.tile_pool(name="ppool", bufs=2, space="PSUM"))
        spool = ctx.enter_context(tc.tile_pool(name="spool", bufs=2, space="PSUM"))
        pvpool = ctx.enter_context(tc.tile_pool(name="pvpool", bufs=2, space="PSUM"))
        dpool = ctx.enter_context(tc.tile_pool(name="dram", bufs=4, space="DRAM"))

        # ---- dummy collective first: syncs the cores and absorbs the
        # collective-stream warmup while the compute phase runs ----
        dummy_i = dpool.tile([8, 16], bf16, name="dummy_i", tag="dummy_i")
        dummy_o = dpool.tile([8, 16], bf16, name="dummy_o", tag="dummy_o")
        nc.gpsimd.collective_compute(
            "AllToAll", ALU.bypass, replica_groups=[list(range(NCORES))],
            ins=[dummy_i.opt()], outs=[dummy_o.opt()])

        # ---- constants on the GpSimd DMA queue so the x loads own Sync ----
        wqT_sb = const.tile([128, NCH, LD], bf16)
        nc.gpsimd.dma_start(wqT_sb, wqT.ap().rearrange("(c p) d -> p c d", p=128))
        wkT_sb = const.tile([128, NCH, LD], bf16)
        nc.gpsimd.dma_start(wkT_sb, wkT.ap().rearrange("(c p) d -> p c d", p=128))
        bq_col = const.tile([128, 1], f32)
        nc.gpsimd.dma_start(bq_col, bq.ap().rearrange("(p o) -> p o", o=1))
        bk_col = const.tile([128, 1], f32)
        nc.gpsimd.dma_start(bk_col, bk.ap().rearrange("(p o) -> p o", o=1))
        wvT_sb = const.tile([128, NCH, LD], bf16)
        nc.gpsimd.dma_start(wvT_sb, wvT.ap().rearrange("(c p) d -> p c d", p=128))
        bv_bc = const.tile([128, LD], f32)
        nc.gpsimd.dma_start(bv_bc, bv.ap().partition_broadcast(128))
        ids_sb = const.tile([128, B * NKT], i32)
        nc.gpsimd.dma_start(ids_sb, ids.ap())
        woT_sb = const.tile([128, NCH, D], bf16)
        nc.gpsimd.dma_start(woT_sb, woT.ap().rearrange("(c p) n -> p c n", p=128))
        bo_bc = const.tile([128, D], f32)
        nc.gpsimd.dma_start(bo_bc, bo.ap().partition_broadcast(128))

        ones64 = const.tile([1, 64], bf16)
        nc.vector.memset(ones64, 1.0)

        # x^T in one [128, c, r] tile; one big DMA per 512-row block so
        # the Sync queue issues 4 descriptors per batch instead of 32
        xTr = xT.ap().rearrange("(c p) r -> p c r", p=128)
        xc = xcp.tile([128, NCH, S], bf16, name="xc", tag="xc")

        def xc_load_rb(b, rb):
            rsl = slice(rb * 512, (rb + 1) * 512)
            dsl = slice(b * S + rb * 512, b * S + (rb + 1) * 512)
            nc.sync.dma_start(xc[:, :, rsl], xTr[:, :, dsl])

        for rb in range(4):
            xc_load_rb(0, rb)

        padf = const.tile([128, B * NKT], f32)
        nc.vector.tensor_copy(padf, ids_sb)
        nc.vector.tensor_scalar_min(padf, padf, 1.0)

        # diagmask2[x, h, y] = 1 if y >= x else 0 (keys on partitions)
        diagmask = const.tile([128, 128], bf16)
        nc.gpsimd.memset(diagmask, 1.0)
        nc.gpsimd.affine_select(
            out=diagmask, in_=diagmask, compare_op=ALU.is_ge, fill=0.0,
            base=0, pattern=[[1, 128]], channel_multiplier=-1,
        )
        diagmask2 = const.tile([128, 2, 128], bf16)
        nc.vector.tensor_copy(diagmask2[:, 0, :], diagmask)
        nc.vector.tensor_copy(diagmask2[:, 1, :], diagmask)

        # ---- per-batch persistent tiles ----
        qt = {}
        kt_ = {}
        vaug = {}
        stage = {}
        ests = {}
        pos = {}
        recbs = {}
        a2a_outs = {}

        EST_BUFS = [2, 2, 1, 1]

        def get_batch_tiles(b):
            if b in qt:
                return
            qt[b] = qkp.tile([128, S], bf16, name=f"qt{b}", tag="qt")
            kt_[b] = qkp.tile([128, S], bf16, name=f"kt{b}", tag="kt")
            vaug[b] = qkp.tile([128, 2, NKT, HD + 1], bf16, name=f"vaug{b}",
                               tag="vaug")
            stage[b] = stg.tile([128, S], bf16, name=f"stage{b}", tag="stage")
            ests[b] = [estp.tile([128, 2, blocklen], bf16, name=f"est{c}",
                                 tag=f"est{c}", bufs=EST_BUFS[c])
                       for c, (_, blocklen) in enumerate(layout)]

        # ---- score entries (the ACT-paced stream) ----
        sc_ready = []
        act_cost = [0.0]
        pe_cost = [0.0]

        mask_pending = {}

        def sc_emit():
            b, c, e = sc_ready.pop(0)
            kt, w, qs, off = layout[c][0][e]
            est = ests[b][c]
            ksl = slice(kt * 128, (kt + 1) * 128)
            ps = spool.tile([128, 2, 512], f32, name="ps", tag="sp")
            nc.tensor.matmul(ps[:, 0, 0:w], kt_[b][0:64, ksl],
                             qt[b][0:64, qs:qs + w], start=True, stop=True)
            nc.tensor.matmul(ps[:, 1, 0:w], kt_[b][64:128, ksl],
                             qt[b][64:128, qs:qs + w], start=True, stop=True)
            nc.scalar.activation(est[:, :, off:off + w], ps[:, :, 0:w],
                                 AF.Exp, scale=0.125)
            if kt >= 4 * c:  # diagonal tile: causal mask, deferred so the
                # exp-gated DVE op doesn't sit ahead of A2A-gating div
                # multiplies in the in-order DVE queue
                mask_pending.setdefault((b, c), []).append(off)
            act_cost[0] += 2 * w * 0.00109 + 0.1

        def emit_masks(b, c):
            est = ests[b][c]
            for off in mask_pending.pop((b, c), []):
                nc.vector.tensor_mul(est[:, :, off:off + 128],
                                     est[:, :, off:off + 128], diagmask2)

        def enq(b, c):
            for e in range(len(layout[c][0])):
                sc_ready.append((b, c, e))

        def pump():
            while sc_ready and act_cost[0] < pe_cost[0] + 4.0:
                sc_emit()

        def force(b, c):
            while sc_ready and sc_ready[0][:2] <= (b, c):
                sc_emit()

        def fill(us, fn, *args):
            fn(*args)
            pe_cost[0] += us
            pump()

        # ---- micro-unit worklets ----
        qk_state = {}

        def qk_unit(b, rb, j):
            """Quarter of a q/k projection row-block: chunks 2j, 2j+1."""
            get_batch_tiles(b)
            rsl = slice(rb * 512, (rb + 1) * 512)
            if j == 0:
                qk_state['pqt'] = ppool.tile([128, 512], f32, name="pqt",
                                             tag="pp")
                qk_state['pkt'] = ppool.tile([128, 512], f32, name="pkt",
                                             tag="pp")
            pqt, pkt = qk_state['pqt'], qk_state['pkt']
            for c in (2 * j, 2 * j + 1):
                st = c == 0
                sp = c == NCH - 1
                rhs = xc[:, c, rsl]
                nc.tensor.matmul(pqt, wqT_sb[:, c, :], rhs, start=st, stop=sp)
                nc.tensor.matmul(pkt, wkT_sb[:, c, :], rhs, start=st, stop=sp)
            if j == 3:
                nc.vector.tensor_scalar_add(qt[b][:, rsl], pqt, bq_col)
                nc.vector.tensor_scalar_add(kt_[b][:, rsl], pkt, bk_col)

        def v_unit(b, m0):
            """Two V m-tiles (keys 128*m0 .. 128*m0+256)."""
            for m in (m0, m0 + 1):
                msl = slice(m * 128, (m + 1) * 128)
                pv_ = ppool.tile([128, LD], f32, name="pv", tag="pp")
                for c in range(NCH):
                    nc.tensor.matmul(pv_, xc[:, c, msl], wvT_sb[:, c, :],
                                     start=(c == 0), stop=(c == NCH - 1))
                tv = work.tile([128, LD], f32, name="tv", tag="tv")
                nc.vector.tensor_add(tv, pv_, bv_bc)
                pcol = padf[:, b * NKT + m:b * NKT + m + 1]
                for h in range(2):
                    nc.vector.tensor_scalar_mul(
                        vaug[b][:, h, m, 0:HD], tv[:, h * HD:(h + 1) * HD],
                        pcol)
                    nc.vector.tensor_copy(vaug[b][:, h, m, HD:HD + 1], pcol)

        def pv(b, c):
            entries, _ = layout[c]
            est = ests[b][c]
            for h in range(2):
                po = pvpool.tile([128, 512], f32, name=f"po{h}", tag="po")
                pos[(b, c, h)] = po
                last = 4 * c + 3
                for kt, w, qs, off in entries:
                    po_off = qs - 512 * c
                    nc.tensor.matmul(po[0:HD + 1, po_off:po_off + w],
                                     vaug[b][:, h, kt, :],
                                     est[:, h, off:off + w],
                                     start=(kt == 0), stop=(kt == last))
                den = recp.tile([1, 512], f32, name="den", tag=f"den{h}")
                nc.vector.tensor_copy(den, po[HD:HD + 1, :])
                rec = recp.tile([1, 512], f32, name="rec", tag=f"rec{h}")
                nc.vector.reciprocal_approx_fast(rec, den)
                recb = recp.tile([1, 512], bf16, name="recb",
                                 tag=f"recb{h}", bufs=2)
                nc.vector.tensor_copy(recb, rec)
                recbs[(b, c, h)] = recb

        def div(b, c):
            for h in range(2):
                nc.tensor.matmul(pos[(b, c, h)][64:128, :], ones64,
                                 recbs[(b, c, h)], start=True, stop=True,
                                 skip_group_check=True)
            for h in range(2):
                po = pos[(b, c, h)]
                rbc = recp.tile([HD, 512], bf16, name="rbc", tag=f"rbc{h}")
                nc.vector.tensor_copy(rbc, po[64:128, :])
                nc.vector.tensor_mul(
                    stage[b][h * HD:(h + 1) * HD, 512 * c:512 * (c + 1)],
                    po[0:HD, :], rbc)

        def a2a(b, h2):
            q0, q1 = 1024 * h2, 1024 * (h2 + 1)
            nq = (q1 - q0) // NCORES
            a2a_in = dpool.tile([NCORES * 128, nq], bf16,
                                name=f"a2ai{b}{h2}", tag="a2ai")
            nc.gpsimd.dma_start(
                a2a_in.rearrange("(j p) r -> p j r", p=128),
                stage[b][:, q0:q1].rearrange("p (j r) -> p j r", j=NCORES))
            a2a_out = dpool.tile([NCORES * 128, nq], bf16,
                                 name=f"a2ao{b}{h2}", tag="a2ao")
            nc.gpsimd.collective_compute(
                "AllToAll", ALU.bypass,
                replica_groups=[list(range(NCORES))],
                ins=[a2a_in.opt()], outs=[a2a_out.opt()])
            a2a_outs[(b, h2)] = a2a_out

        op_state = {}

        def op_unit(b, h2, n):
            """Half an output-projection chunk (512 of 1024 out dims)."""
            if n == 0:
                a2a_sb = stg.tile([128, NCORES, 128], bf16,
                                  name=f"a2as{b}{h2}", tag="a2as", bufs=3)
                nc.sync.dma_start(
                    a2a_sb,
                    a2a_outs[(b, h2)].rearrange("(j p) r -> p j r", p=128))
                op_state[(b, h2)] = a2a_sb
            a2a_sb = op_state[(b, h2)]
            r0 = (2 * b + h2) * 128
            pout = ppool.tile([128, 512], f32, name="pout", tag="pp")
            for c in range(NCH):
                nc.tensor.matmul(
                    pout, a2a_sb[:, c, :],
                    woT_sb[:, c, n * 512:(n + 1) * 512],
                    start=(c == 0), stop=(c == NCH - 1))
            ot = work.tile([128, 512], f32, name="ot", tag="ot")
            nc.vector.tensor_add(ot, pout, bo_bc[:, n * 512:(n + 1) * 512])
            nc.sync.dma_start(
                out.ap()[r0:r0 + 128, n * 512:(n + 1) * 512], ot)

        # ---- emission schedule ----
        QKU, VU, OPU, DIVU = 1.05, 1.1, 2.1, 0.55

        # batch-0 projections: V units follow each qk row-block so the
        # xc region's batch-1 reload (issued right behind, Sync queue)
        # unblocks as early as possible
        for rb in range(4):
            for j in range(4):
                fill(QKU, qk_unit, 0, rb, j)
            enq(0, rb)
            fill(VU, v_unit, 0, 4 * rb)
            fill(VU, v_unit, 0, 4 * rb + 2)
            xc_load_rb(1, rb)

        def div_a2a(b, c, h2=None):
            div(b, c)
            if h2 is not None:
                a2a(b, h2)
            pe_cost[0] += DIVU
            pump()

        def pv_stripe(us, b, c):
            force(b, c)
            emit_masks(b, c)
            fill(us, pv, b, c)

        for j in range(4):
            fill(QKU, qk_unit, 1, 0, j)
        enq(1, 0)
        pv_stripe(1.3, 0, 0)
        for j in range(4):
            fill(QKU, qk_unit, 1, 1, j)
        enq(1, 1)
        div_a2a(0, 0)
        pv_stripe(3.4, 0, 1)
        for j in range(4):
            fill(QKU, qk_unit, 1, 2, j)
        enq(1, 2)
        div_a2a(0, 1, 0)
        pv_stripe(5.5, 0, 2)
        for j in range(4):
            fill(QKU, qk_unit, 1, 3, j)
        enq(1, 3)
        div_a2a(0, 2)
        for m0 in range(0, 6, 2):
            fill(VU, v_unit, 1, m0)
        pv_stripe(7.6, 0, 3)
        fill(VU, v_unit, 1, 6)
        div_a2a(0, 3, 1)
        for m0 in range(8, NKT, 2):
            fill(VU, v_unit, 1, m0)
        pv_stripe(1.3, 1, 0)
        div_a2a(1, 0)
        pv_stripe(3.4, 1, 1)
        div_a2a(1, 1, 0)
        pv_stripe(5.5, 1, 2)
        div_a2a(1, 2)
        fill(OPU, op_unit, 0, 0, 0)
        fill(OPU, op_unit, 0, 0, 1)
        fill(OPU, op_unit, 0, 1, 0)
        fill(OPU, op_unit, 0, 1, 1)
        pv_stripe(7.6, 1, 3)
        div_a2a(1, 3, 1)
        fill(OPU, op_unit, 1, 0, 0)
        fill(OPU, op_unit, 1, 0, 1)
        fill(OPU, op_unit, 1, 1, 0)
        fill(OPU, op_unit, 1, 1, 1)

        assert not sc_ready
        assert not mask_pending

    nc.finalize()
    return nc


def _get_nc():
    with _lock:
        if "nc" not in _cache:
            _cache["nc"] = _build_nc()
        return _cache["nc"]


def _shard_inputs(x, input_ids, Wq, bq, Wk, bk, Wv, bv, Wo, bo):
    import ml_dtypes
    bf16 = ml_dtypes.bfloat16

    x = np.asarray(x, dtype=np.float32)
    xT = np.ascontiguousarray(x.reshape(R, D).T).astype(bf16)
    woT = np.ascontiguousarray(np.asarray(Wo, dtype=np.float32).T).astype(bf16)
    bo_f = np.asarray(bo, dtype=np.float32)
    ids = np.asarray(input_ids).astype(np.int32)
    # ids_r[p, b*NKT + t] = input_ids[b, t*128 + p]
    ids_r = np.ascontiguousarray(ids.reshape(B, NKT, 128).transpose(2, 0, 1)
                                 .reshape(128, B * NKT))
    Wq = np.asarray(Wq, dtype=np.float32)
    Wk = np.asarray(Wk, dtype=np.float32)
    Wv = np.asarray(Wv, dtype=np.float32)
    bq = np.asarray(bq, dtype=np.float32)
    bk = np.asarray(bk, dtype=np.float32)
    bv = np.asarray(bv, dtype=np.float32)

    in_maps = []
    for c in range(NCORES):
        sl = slice(c * LD, (c + 1) * LD)
        in_maps.append({
            "xT": xT,
            "wqT": np.ascontiguousarray(Wq[sl].T).astype(bf16),
            "wkT": np.ascontiguousarray(Wk[sl].T).astype(bf16),
            "wvT": np.ascontiguousarray(Wv[sl].T).astype(bf16),
            "woT": woT,
            "bq": bq[sl].copy(),
            "bk": bk[sl].copy(),
            "bv": bv[sl].copy(),
            "bo": bo_f,
            "ids": ids_r,
        })
    return in_maps


def run(trace=False, **inputs):
    """Run the kernel; returns (output, BassKernelResults)."""
    from concourse.bass_utils import run_bass_kernel_spmd

    nc = _get_nc()
    in_maps = _shard_inputs(**inputs)
    res = run_bass_kernel_spmd(nc, in_maps, core_ids=list(range(NCORES)),
                               trace=trace)
    full = np.empty((B, S, D), dtype=np.float32)
    for j in range(NCORES):
        o = np.asarray(res.results[j]["out"], dtype=np.float32)
        for b in range(B):
            for h2 in range(2):
                full[b, 1024 * h2 + 128 * j:1024 * h2 + 128 * (j + 1), :] = \
                    o[(2 * b + h2) * 128:(2 * b + h2 + 1) * 128, :]
    return full, res


def kernel(**inputs) -> np.ndarray:
    full, _ = run(trace=False, **inputs)
    return full


# revision 15
# speedup vs baseline: 1.0415x; 1.0415x over previous
"""Trainium2 Bass kernel for a causal multi-head attention layer.

Model: b=2, s=2048, d_model=1024, 16 heads, head_dim=64, pad-index 0.
Sharding over 8 NeuronCores: each core owns 2 heads (128 of the 1024
attention dims) for both batches (head/tensor parallel).  After attention,
AllToAlls redistribute the per-head outputs so each core holds all 1024
attention dims for 1/8 of the sequence positions, where it runs the output
projection locally.  Output rows per core: 512 (4 chunks of 128).

Schedule: the exp of the attention scores (ACT engine, ~88us for both
batches) is the critical chain, and the PE stream blocks on the 2-deep
score-PSUM ring whenever it runs more than 2 score entries ahead of the
ACT engine.  So the emitter rate-matches: score entries are woven one at
a time between ~1us micro-units of projection / PV / output-projection
work.  A2As run as 4 collectives (one per half-batch) fired as soon as
each half is normalized; a dummy collective at the top absorbs the
collective-stream warmup and core-launch skew.
"""

import threading

import numpy as np

B, S, D = 2, 2048, 1024
H, HD = 16, 64
NCORES = 8
LD = D // NCORES          # 128 local attention dims (2 heads)
R = B * S                 # 4096 flattened rows
RC = R // NCORES          # 512 output rows per core
NKT = S // 128            # 16 key tiles per batch
NCH = D // 128            # 8 contraction chunks of d_model
NST = S // 512            # 4 query stripes per batch

_cache = {}
_lock = threading.Lock()


def _stripe_layout():
    """Per stripe c: list of (kt, width, q_start, offset-in-block), block len."""
    layout = []
    for c in range(NST):
        entries = []
        off = 0
        for kt in range(4 * c + 4):
            qs = max(512 * c, kt * 128)
            w = 512 * (c + 1) - qs
            entries.append((kt, w, qs, off))
            off += w
        layout.append((entries, off))
    return layout


def _build_nc():
    import concourse.mybir as mybir
    import concourse.tile as tile
    from concourse import bacc
    from contextlib import ExitStack

    f32 = mybir.dt.float32
    bf16 = mybir.dt.bfloat16
    i32 = mybir.dt.int32
    AF = mybir.ActivationFunctionType
    ALU = mybir.AluOpType

    nc = bacc.Bacc(None, target_bir_lowering=False, num_devices=NCORES)

    xT = nc.declare_dram_parameter("xT", [D, R], bf16, isOutput=False)
    wqT = nc.declare_dram_parameter("wqT", [D, LD], bf16, isOutput=False)
    wkT = nc.declare_dram_parameter("wkT", [D, LD], bf16, isOutput=False)
    wvT = nc.declare_dram_parameter("wvT", [D, LD], bf16, isOutput=False)
    woT = nc.declare_dram_parameter("woT", [D, D], bf16, isOutput=False)
    bq = nc.declare_dram_parameter("bq", [LD], f32, isOutput=False)
    bk = nc.declare_dram_parameter("bk", [LD], f32, isOutput=False)
    bv = nc.declare_dram_parameter("bv", [LD], f32, isOutput=False)
    bo = nc.declare_dram_parameter("bo", [D], f32, isOutput=False)
    ids = nc.declare_dram_parameter("ids", [128, B * NKT], i32, isOutput=False)
    out = nc.declare_dram_parameter("out", [RC, D], f32, isOutput=True)

    layout = _stripe_layout()

    with ExitStack() as ctx:
        tc = ctx.enter_context(tile.TileContext(nc))
        const = ctx.enter_context(tc.tile_pool(name="const", bufs=1))
        xcp = ctx.enter_context(tc.tile_pool(name="xcp", bufs=1))
        qkp = ctx.enter_context(tc.tile_pool(name="qkp", bufs=2))
        estp = ctx.enter_context(tc.tile_pool(name="estp", bufs=1))
        stg = ctx.enter_context(tc.tile_pool(name="stg", bufs=2))
        work = ctx.enter_context(tc.tile_pool(name="work", bufs=2))
        recp = ctx.enter_context(tc.tile_pool(name="recp", bufs=1))
        ppool = ctx.enter_context(tc.tile_pool(name="ppool", bufs=2, space="PSUM"))
        spool = ctx.enter_context(tc.tile_pool(name="spool", bufs=2, space="PSUM"))
        pvpool = ctx.enter_context(tc.tile_pool(name="pvpool", bufs=2, space="PSUM"))
        dpool = ctx.enter_context(tc.tile_pool(name="dram", bufs=4, space="DRAM"))

        # ---- dummy collective first: syncs the cores and absorbs the
        # collective-stream warmup while the compute phase runs ----
        dummy_i = dpool.tile([8, 16], bf16, name="dummy_i", tag="dummy_i")
        dummy_o = dpool.tile([8, 16], bf16, name="dummy_o", tag="dummy_o")
        nc.gpsimd.collective_compute(
            "AllToAll", ALU.bypass, replica_groups=[list(range(NCORES))],
            ins=[dummy_i.opt()], outs=[dummy_o.opt()])

        # ---- constants on the GpSimd DMA queue so the x loads own Sync ----
        wqT_sb = const.tile([128, NCH, LD], bf16)
        nc.gpsimd.dma_start(wqT_sb, wqT.ap().rearrange("(c p) d -> p c d", p=128))
        wkT_sb = const.tile([128, NCH, LD], bf16)
        nc.gpsimd.dma_start(wkT_sb, wkT.ap().rearrange("(c p) d -> p c d", p=128))
        bq_col = const.tile([128, 1], f32)
        nc.gpsimd.dma_start(bq_col, bq.ap().rearrange("(p o) -> p o", o=1))
        bk_col = const.tile([128, 1], f32)
        nc.gpsimd.dma_start(bk_col, bk.ap().rearrange("(p o) -> p o", o=1))
        wvT_sb = const.tile([128, NCH, LD], bf16)
        nc.gpsimd.dma_start(wvT_sb, wvT.ap().rearrange("(c p) d -> p c d", p=128))
        bv_bc = const.tile([128, LD], f32)
        nc.gpsimd.dma_start(bv_bc, bv.ap().partition_broadcast(128))
        ids_sb = const.tile([128, B * NKT], i32)
        nc.gpsimd.dma_start(ids_sb, ids.ap())
        woT_sb = const.tile([128, NCH, D], bf16)
        nc.gpsimd.dma_start(woT_sb, woT.ap().rearrange("(c p) n -> p c n", p=128))
        bo_bc = const.tile([128, D], f32)
        nc.gpsimd.dma_start(bo_bc, bo.ap().partition_broadcast(128))

        ones64 = const.tile([1, 64], bf16)
        nc.vector.memset(ones64, 1.0)

        # x^T in one [128, c, r] tile; one big DMA per 512-row block so
        # the Sync queue issues 4 descriptors per batch instead of 32
        xTr = xT.ap().rearrange("(c p) r -> p c r", p=128)
        xc = xcp.tile([128, NCH, S], bf16, name="xc", tag="xc")

        def xc_load_rb(b, rb):
            rsl = slice(rb * 512, (rb + 1) * 512)
            dsl = slice(b * S + rb * 512, b * S + (rb + 1) * 512)
            nc.sync.dma_start(xc[:, :, rsl], xTr[:, :, dsl])

        for rb in range(4):
            xc_load_rb(0, rb)

        padf = const.tile([128, B * NKT], f32)
        nc.vector.tensor_copy(padf, ids_sb)
        nc.vector.tensor_scalar_min(padf, padf, 1.0)

        # diagmask2[x, h, y] = 1 if y >= x else 0 (keys on partitions)
        diagmask = const.tile([128, 128], bf16)
        nc.gpsimd.memset(diagmask, 1.0)
        nc.gpsimd.affine_select(
            out=diagmask, in_=diagmask, compare_op=ALU.is_ge, fill=0.0,
            base=0, pattern=[[1, 128]], channel_multiplier=-1,
        )
        diagmask2 = const.tile([128, 2, 128], bf16)
        nc.vector.tensor_copy(diagmask2[:, 0, :], diagmask)
        nc.vector.tensor_copy(diagmask2[:, 1, :], diagmask)

        # ---- per-batch persistent tiles ----
        qt = {}
        kt_ = {}
        vaug = {}
        stage = {}
        ests = {}
        pos = {}
        recbs = {}
        a2a_outs = {}

        EST_BUFS = [2, 2, 1, 1]

        def get_batch_tiles(b):
            if b in qt:
                return
            qt[b] = qkp.tile([128, S], bf16, name=f"qt{b}", tag="qt")
            kt_[b] = qkp.tile([128, S], bf16, name=f"kt{b}", tag="kt")
            vaug[b] = qkp.tile([128, 2, NKT, HD + 1], bf16, name=f"vaug{b}",
                               tag="vaug")
            stage[b] = stg.tile([128, S], bf16, name=f"stage{b}", tag="stage")
            ests[b] = [estp.tile([128, 2, blocklen], bf16, name=f"est{c}",
                                 tag=f"est{c}", bufs=EST_BUFS[c])
                       for c, (_, blocklen) in enumerate(layout)]

        # ---- score entries (the ACT-paced stream) ----
        sc_ready = []
        act_cost = [0.0]
        pe_cost = [0.0]

        mask_pending = {}

        def sc_emit():
            b, c, e = sc_ready.pop(0)
            kt, w, qs, off = layout[c][0][e]
            est = ests[b][c]
            ksl = slice(kt * 128, (kt + 1) * 128)
            ps = spool.tile([128, 2, 512], f32, name="ps", tag="sp")
            nc.tensor.matmul(ps[:, 0, 0:w], kt_[b][0:64, ksl],
                             qt[b][0:64, qs:qs + w], start=True, stop=True)
            nc.tensor.matmul(ps[:, 1, 0:w], kt_[b][64:128, ksl],
                             qt[b][64:128, qs:qs + w], start=True, stop=True)
            nc.scalar.activation(est[:, :, off:off + w], ps[:, :, 0:w],
                                 AF.Exp, scale=0.125)
            if kt >= 4 * c:  # diagonal tile: causal mask, deferred so the
                # exp-gated DVE op doesn't sit ahead of A2A-gating div
                # multiplies in the in-order DVE queue
                mask_pending.setdefault((b, c), []).append(off)
            act_cost[0] += 2 * w * 0.00109 + 0.1

        def emit_masks(b, c):
            est = ests[b][c]
            for off in mask_pending.pop((b, c), []):
                nc.vector.tensor_mul(est[:, :, off:off + 128],
                                     est[:, :, off:off + 128], diagmask2)

        def enq(b, c):
            for e in range(len(layout[c][0])):
                sc_ready.append((b, c, e))

        greedy = [False]

        def pump():
            while sc_ready and (greedy[0]
                                or act_cost[0] < pe_cost[0] + 4.0):
                sc_emit()

        def force(b, c):
            while sc_ready and sc_ready[0][:2] <= (b, c):
                sc_emit()

        def fill(us, fn, *args):
            fn(*args)
            pe_cost[0] += us
            pump()

        # ---- micro-unit worklets ----
        qk_state = {}

        def qk_unit(b, rb, j):
            """Quarter of a q/k projection row-block: chunks 2j, 2j+1."""
            get_batch_tiles(b)
            rsl = slice(rb * 512, (rb + 1) * 512)
            if j == 0:
                qk_state['pqt'] = ppool.tile([128, 512], f32, name="pqt",
                                             tag="pp")
                qk_state['pkt'] = ppool.tile([128, 512], f32, name="pkt",
                                             tag="pp")
            pqt, pkt = qk_state['pqt'], qk_state['pkt']
            for c in (2 * j, 2 * j + 1):
                st = c == 0
                sp = c == NCH - 1
                rhs = xc[:, c, rsl]
                nc.tensor.matmul(pqt, wqT_sb[:, c, :], rhs, start=st, stop=sp)
                nc.tensor.matmul(pkt, wkT_sb[:, c, :], rhs, start=st, stop=sp)
            if j == 3:
                nc.vector.tensor_scalar_add(qt[b][:, rsl], pqt, bq_col)
                nc.vector.tensor_scalar_add(kt_[b][:, rsl], pkt, bk_col)

        def v_unit(b, m0):
            """Two V m-tiles (keys 128*m0 .. 128*m0+256)."""
            for m in (m0, m0 + 1):
                msl = slice(m * 128, (m + 1) * 128)
                pv_ = ppool.tile([128, LD], f32, name="pv", tag="pp")
                for c in range(NCH):
                    nc.tensor.matmul(pv_, xc[:, c, msl], wvT_sb[:, c, :],
                                     start=(c == 0), stop=(c == NCH - 1))
                tv = work.tile([128, LD], f32, name="tv", tag="tv")
                nc.vector.tensor_add(tv, pv_, bv_bc)
                pcol = padf[:, b * NKT + m:b * NKT + m + 1]
                for h in range(2):
                    nc.vector.tensor_scalar_mul(
                        vaug[b][:, h, m, 0:HD], tv[:, h * HD:(h + 1) * HD],
                        pcol)
                    nc.vector.tensor_copy(vaug[b][:, h, m, HD:HD + 1], pcol)

        def pv(b, c):
            entries, _ = layout[c]
            est = ests[b][c]
            for h in range(2):
                po = pvpool.tile([128, 512], f32, name=f"po{h}", tag="po")
                pos[(b, c, h)] = po
                last = 4 * c + 3
                for kt, w, qs, off in entries:
                    po_off = qs - 512 * c
                    nc.tensor.matmul(po[0:HD + 1, po_off:po_off + w],
                                     vaug[b][:, h, kt, :],
                                     est[:, h, off:off + w],
                                     start=(kt == 0), stop=(kt == last))
                den = recp.tile([1, 512], f32, name="den", tag=f"den{h}")
                nc.vector.tensor_copy(den, po[HD:HD + 1, :])
                rec = recp.tile([1, 512], f32, name="rec", tag=f"rec{h}")
                nc.vector.reciprocal_approx_fast(rec, den)
                recb = recp.tile([1, 512], bf16, name="recb",
                                 tag=f"recb{h}", bufs=2)
                nc.vector.tensor_copy(recb, rec)
                recbs[(b, c, h)] = recb

        def div(b, c):
            for h in range(2):
                nc.tensor.matmul(pos[(b, c, h)][64:128, :], ones64,
                                 recbs[(b, c, h)], start=True, stop=True,
                                 skip_group_check=True)
            for h in range(2):
                po = pos[(b, c, h)]
                rbc = recp.tile([HD, 512], bf16, name="rbc", tag=f"rbc{h}")
                nc.vector.tensor_copy(rbc, po[64:128, :])
                nc.vector.tensor_mul(
                    stage[b][h * HD:(h + 1) * HD, 512 * c:512 * (c + 1)],
                    po[0:HD, :], rbc)

        def a2a(b, h2):
            q0, q1 = 1024 * h2, 1024 * (h2 + 1)
            nq = (q1 - q0) // NCORES
            a2a_in = dpool.tile([NCORES * 128, nq], bf16,
                                name=f"a2ai{b}{h2}", tag="a2ai")
            nc.gpsimd.dma_start(
                a2a_in.rearrange("(j p) r -> p j r", p=128),
                stage[b][:, q0:q1].rearrange("p (j r) -> p j r", j=NCORES))
            a2a_out = dpool.tile([NCORES * 128, nq], bf16,
                                 name=f"a2ao{b}{h2}", tag="a2ao")
            nc.gpsimd.collective_compute(
                "AllToAll", ALU.bypass,
                replica_groups=[list(range(NCORES))],
                ins=[a2a_in.opt()], outs=[a2a_out.opt()])
            a2a_outs[(b, h2)] = a2a_out

        op_state = {}

        def op_unit(b, h2, n):
            """Half an output-projection chunk (512 of 1024 out dims)."""
            if n == 0:
                a2a_sb = stg.tile([128, NCORES, 128], bf16,
                                  name=f"a2as{b}{h2}", tag="a2as", bufs=3)
                nc.sync.dma_start(
                    a2a_sb,
                    a2a_outs[(b, h2)].rearrange("(j p) r -> p j r", p=128))
                op_state[(b, h2)] = a2a_sb
            a2a_sb = op_state[(b, h2)]
            r0 = (2 * b + h2) * 128
            pout = ppool.tile([128, 512], f32, name="pout", tag="pp")
            for c in range(NCH):
                nc.tensor.matmul(
                    pout, a2a_sb[:, c, :],
                    woT_sb[:, c, n * 512:(n + 1) * 512],
                    start=(c == 0), stop=(c == NCH - 1))
            ot = work.tile([128, 512], f32, name="ot", tag="ot")
            nc.vector.tensor_add(ot, pout, bo_bc[:, n * 512:(n + 1) * 512])
            nc.sync.dma_start(
                out.ap()[r0:r0 + 128, n * 512:(n + 1) * 512], ot)

        # ---- emission schedule ----
        QKU, VU, OPU, DIVU = 1.05, 1.1, 2.1, 0.55

        # batch-0 projections: V units follow each qk row-block so the
        # xc region's batch-1 reload (issued right behind, Sync queue)
        # unblocks as early as possible
        for rb in range(4):
            for j in range(4):
                fill(QKU, qk_unit, 0, rb, j)
            enq(0, rb)
            fill(VU, v_unit, 0, 4 * rb)
            fill(VU, v_unit, 0, 4 * rb + 2)
            xc_load_rb(1, rb)

        def div_a2a(b, c, h2=None):
            div(b, c)
            if h2 is not None:
                a2a(b, h2)
            pe_cost[0] += DIVU
            pump()

        def pv_stripe(us, b, c):
            force(b, c)
            emit_masks(b, c)
            fill(us, pv, b, c)

        for j in range(4):
            fill(QKU, qk_unit, 1, 0, j)
        enq(1, 0)
        pv_stripe(1.3, 0, 0)
        for j in range(4):
            fill(QKU, qk_unit, 1, 1, j)
        enq(1, 1)
        div_a2a(0, 0)
        pv_stripe(3.4, 0, 1)
        for j in range(4):
            fill(QKU, qk_unit, 1, 2, j)
        enq(1, 2)
        div_a2a(0, 1, 0)
        pv_stripe(5.5, 0, 2)
        for j in range(4):
            fill(QKU, qk_unit, 1, 3, j)
        enq(1, 3)
        div_a2a(0, 2)
        for m0 in range(0, 6, 2):
            fill(VU, v_unit, 1, m0)
        pv_stripe(7.6, 0, 3)
        fill(VU, v_unit, 1, 6)
        div_a2a(0, 3, 1)
        for m0 in range(8, NKT, 2):
            fill(VU, v_unit, 1, m0)
        pv_stripe(1.3, 1, 0)
        div_a2a(1, 0)
        pv_stripe(3.4, 1, 1)
        div_a2a(1, 1, 0)
        greedy[0] = True
        pump()
        pv_stripe(5.5, 1, 2)
        div_a2a(1, 2)
        pv_stripe(7.6, 1, 3)
        div_a2a(1, 3, 1)
        fill(OPU, op_unit, 0, 0, 0)
        fill(OPU, op_unit, 0, 0, 1)
        fill(OPU, op_unit, 0, 1, 0)
        fill(OPU, op_unit, 0, 1, 1)
        fill(OPU, op_unit, 1, 0, 0)
        fill(OPU, op_unit, 1, 0, 1)
        fill(OPU, op_unit, 1, 1, 0)
        fill(OPU, op_unit, 1, 1, 1)

        assert not sc_ready
        assert not mask_pending

    nc.finalize()
    return nc


def _get_nc():
    with _lock:
        if "nc" not in _cache:
            _cache["nc"] = _build_nc()
        return _cache["nc"]


def _shard_inputs(x, input_ids, Wq, bq, Wk, bk, Wv, bv, Wo, bo):
    import ml_dtypes
    bf16 = ml_dtypes.bfloat16

    x = np.asarray(x, dtype=np.float32)
    xT = np.ascontiguousarray(x.reshape(R, D).T).astype(bf16)
    woT = np.ascontiguousarray(np.asarray(Wo, dtype=np.float32).T).astype(bf16)
    bo_f = np.asarray(bo, dtype=np.float32)
    ids = np.asarray(input_ids).astype(np.int32)
    # ids_r[p, b*NKT + t] = input_ids[b, t*128 + p]
    ids_r = np.ascontiguousarray(ids.reshape(B, NKT, 128).transpose(2, 0, 1)
                                 .reshape(128, B * NKT))
    Wq = np.asarray(Wq, dtype=np.float32)
    Wk = np.asarray(Wk, dtype=np.float32)
    Wv = np.asarray(Wv, dtype=np.float32)
    bq = np.asarray(bq, dtype=np.float32)
    bk = np.asarray(bk, dtype=np.float32)
    bv = np.asarray(bv, dtype=np.float32)

    in_maps = []
    for c in range(NCORES):
        sl = slice(c * LD, (c + 1) * LD)
        in_maps.append({
            "xT": xT,
            "wqT": np.ascontiguousarray(Wq[sl].T).astype(bf16),
            "wkT": np.ascontiguousarray(Wk[sl].T).astype(bf16),
            "wvT": np.ascontiguousarray(Wv[sl].T).astype(bf16),
            "woT": woT,
            "bq": bq[sl].copy(),
            "bk": bk[sl].copy(),
            "bv": bv[sl].copy(),
            "bo": bo_f,
            "ids": ids_r,
        })
    return in_maps


def run(trace=False, **inputs):
    """Run the kernel; returns (output, BassKernelResults)."""
    from concourse.bass_utils import run_bass_kernel_spmd

    nc = _get_nc()
    in_maps = _shard_inputs(**inputs)
    res = run_bass_kernel_spmd(nc, in_maps, core_ids=list(range(NCORES)),
                               trace=trace)
    full = np.empty((B, S, D), dtype=np.float32)
    for j in range(NCORES):
        o = np.asarray(res.results[j]["out"], dtype=np.float32)
        for b in range(B):
            for h2 in range(2):
                full[b, 1024 * h2 + 128 * j:1024 * h2 + 128 * (j + 1), :] = \
                    o[(2 * b + h2) * 128:(2 * b + h2 + 1) * 128, :]
    return full, res


def kernel(**inputs) -> np.ndarray:
    full, _ = run(trace=False, **inputs)
    return full


# revision 19
# speedup vs baseline: 1.0753x; 1.0325x over previous
"""Trainium2 Bass kernel for a causal multi-head attention layer.

Model: b=2, s=2048, d_model=1024, 16 heads, head_dim=64, pad-index 0.
Sharding over 8 NeuronCores: each core owns 2 heads (128 of the 1024
attention dims) for both batches (head/tensor parallel).  After attention,
AllToAlls redistribute the per-head outputs so each core holds all 1024
attention dims for 1/8 of the sequence positions, where it runs the output
projection locally.  Output rows per core: 512 (4 chunks of 128).

Schedule: the exp of the attention scores (ACT engine, ~88us for both
batches) is the critical chain, and the PE stream blocks on the 2-deep
score-PSUM ring whenever it runs more than 2 score entries ahead of the
ACT engine.  So the emitter rate-matches: score entries are woven one at
a time between ~1us micro-units of projection / PV / output-projection
work.  A2As run as 4 collectives (one per half-batch) fired as soon as
each half is normalized; a dummy collective at the top absorbs the
collective-stream warmup and core-launch skew.
"""

import threading

import numpy as np

B, S, D = 2, 2048, 1024
H, HD = 16, 64
NCORES = 8
LD = D // NCORES          # 128 local attention dims (2 heads)
R = B * S                 # 4096 flattened rows
RC = R // NCORES          # 512 output rows per core
NKT = S // 128            # 16 key tiles per batch
NCH = D // 128            # 8 contraction chunks of d_model
NST = S // 512            # 4 query stripes per batch

_cache = {}
_lock = threading.Lock()


def _stripe_layout():
    """Per stripe c: list of (kt, width, q_start, offset-in-block), block len."""
    layout = []
    for c in range(NST):
        entries = []
        off = 0
        for kt in range(4 * c + 4):
            qs = max(512 * c, kt * 128)
            w = 512 * (c + 1) - qs
            entries.append((kt, w, qs, off))
            off += w
        layout.append((entries, off))
    return layout


def _build_nc():
    import concourse.mybir as mybir
    import concourse.tile as tile
    from concourse import bacc
    from contextlib import ExitStack

    f32 = mybir.dt.float32
    bf16 = mybir.dt.bfloat16
    i32 = mybir.dt.int32
    AF = mybir.ActivationFunctionType
    ALU = mybir.AluOpType

    nc = bacc.Bacc(None, target_bir_lowering=False, num_devices=NCORES)

    xT = nc.declare_dram_parameter("xT", [D, R], bf16, isOutput=False)
    wqT = nc.declare_dram_parameter("wqT", [D, LD], bf16, isOutput=False)
    wkT = nc.declare_dram_parameter("wkT", [D, LD], bf16, isOutput=False)
    wvT = nc.declare_dram_parameter("wvT", [D, LD], bf16, isOutput=False)
    woT = nc.declare_dram_parameter("woT", [D, D], bf16, isOutput=False)
    bq = nc.declare_dram_parameter("bq", [LD], f32, isOutput=False)
    bk = nc.declare_dram_parameter("bk", [LD], f32, isOutput=False)
    bv = nc.declare_dram_parameter("bv", [LD], f32, isOutput=False)
    bo = nc.declare_dram_parameter("bo", [D], f32, isOutput=False)
    ids = nc.declare_dram_parameter("ids", [128, B * NKT], i32, isOutput=False)
    out = nc.declare_dram_parameter("out", [RC, D], f32, isOutput=True)

    layout = _stripe_layout()

    with ExitStack() as ctx:
        tc = ctx.enter_context(tile.TileContext(nc))
        const = ctx.enter_context(tc.tile_pool(name="const", bufs=1))
        xcp = ctx.enter_context(tc.tile_pool(name="xcp", bufs=1))
        qkp = ctx.enter_context(tc.tile_pool(name="qkp", bufs=2))
        estp = ctx.enter_context(tc.tile_pool(name="estp", bufs=1))
        stg = ctx.enter_context(tc.tile_pool(name="stg", bufs=2))
        work = ctx.enter_context(tc.tile_pool(name="work", bufs=2))
        recp = ctx.enter_context(tc.tile_pool(name="recp", bufs=1))
        ppool = ctx.enter_context(tc.tile_pool(name="ppool", bufs=2, space="PSUM"))
        spool = ctx.enter_context(tc.tile_pool(name="spool", bufs=2, space="PSUM"))
        pvpool = ctx.enter_context(tc.tile_pool(name="pvpool", bufs=2, space="PSUM"))
        dpool = ctx.enter_context(tc.tile_pool(name="dram", bufs=4, space="DRAM"))

        # ---- dummy collective first: syncs the cores and absorbs the
        # collective-stream warmup while the compute phase runs ----
        dummy_i = dpool.tile([8, 16], bf16, name="dummy_i", tag="dummy_i")
        dummy_o = dpool.tile([8, 16], bf16, name="dummy_o", tag="dummy_o")
        nc.gpsimd.collective_compute(
            "AllToAll", ALU.bypass, replica_groups=[list(range(NCORES))],
            ins=[dummy_i.opt()], outs=[dummy_o.opt()])

        # ---- constants on the GpSimd DMA queue so the x loads own Sync ----
        wqT_sb = const.tile([128, NCH, LD], bf16)
        nc.gpsimd.dma_start(wqT_sb, wqT.ap().rearrange("(c p) d -> p c d", p=128))
        wkT_sb = const.tile([128, NCH, LD], bf16)
        nc.gpsimd.dma_start(wkT_sb, wkT.ap().rearrange("(c p) d -> p c d", p=128))
        bq_col = const.tile([128, 1], f32)
        nc.gpsimd.dma_start(bq_col, bq.ap().rearrange("(p o) -> p o", o=1))
        bk_col = const.tile([128, 1], f32)
        nc.gpsimd.dma_start(bk_col, bk.ap().rearrange("(p o) -> p o", o=1))
        wvT_sb = const.tile([128, NCH, LD], bf16)
        nc.gpsimd.dma_start(wvT_sb, wvT.ap().rearrange("(c p) d -> p c d", p=128))
        bv_bc = const.tile([128, LD], f32)
        nc.gpsimd.dma_start(bv_bc, bv.ap().partition_broadcast(128))
        ids_sb = const.tile([128, B * NKT], i32)
        nc.gpsimd.dma_start(ids_sb, ids.ap())
        woT_sb = const.tile([128, NCH, D], bf16)
        nc.gpsimd.dma_start(woT_sb, woT.ap().rearrange("(c p) n -> p c n", p=128))
        bo_bc = const.tile([128, D], f32)
        nc.gpsimd.dma_start(bo_bc, bo.ap().partition_broadcast(128))

        ones64 = const.tile([1, 64], bf16)
        nc.vector.memset(ones64, 1.0)

        # warm-up matmuls: keep the PE HAM busy through the startup DMA
        # window so the first projections run at the full clock
        warm_ps = pvpool.tile([64, 64], f32, name="warm", tag="po")
        for _ in range(90):
            nc.tensor.matmul(warm_ps, ones64, ones64, start=True, stop=True)

        # x^T in one [128, c, r] tile; one big DMA per 512-row block so
        # the Sync queue issues 4 descriptors per batch instead of 32
        xTr = xT.ap().rearrange("(c p) r -> p c r", p=128)
        xc = xcp.tile([128, NCH, S], bf16, name="xc", tag="xc")

        def xc_load_rb(b, rb):
            rsl = slice(rb * 512, (rb + 1) * 512)
            dsl = slice(b * S + rb * 512, b * S + (rb + 1) * 512)
            nc.sync.dma_start(xc[:, :, rsl], xTr[:, :, dsl])

        for rb in range(4):
            xc_load_rb(0, rb)

        padf = const.tile([128, B * NKT], f32)
        nc.vector.tensor_copy(padf, ids_sb)
        nc.vector.tensor_scalar_min(padf, padf, 1.0)

        # diagmask2[x, h, y] = 1 if y >= x else 0 (keys on partitions)
        diagmask = const.tile([128, 128], bf16)
        nc.gpsimd.memset(diagmask, 1.0)
        nc.gpsimd.affine_select(
            out=diagmask, in_=diagmask, compare_op=ALU.is_ge, fill=0.0,
            base=0, pattern=[[1, 128]], channel_multiplier=-1,
        )
        diagmask2 = const.tile([128, 2, 128], bf16)
        nc.vector.tensor_copy(diagmask2[:, 0, :], diagmask)
        nc.vector.tensor_copy(diagmask2[:, 1, :], diagmask)

        # ---- per-batch persistent tiles ----
        qt = {}
        kt_ = {}
        vaug = {}
        stage = {}
        ests = {}
        pos = {}
        recbs = {}
        a2a_outs = {}

        EST_BUFS = [2, 2, 1, 1]

        def get_batch_tiles(b):
            if b in qt:
                return
            qt[b] = qkp.tile([128, S], bf16, name=f"qt{b}", tag="qt")
            kt_[b] = qkp.tile([128, S], bf16, name=f"kt{b}", tag="kt")
            vaug[b] = qkp.tile([128, 2, NKT, HD + 1], bf16, name=f"vaug{b}",
                               tag="vaug")
            stage[b] = stg.tile([128, S], bf16, name=f"stage{b}", tag="stage")
            ests[b] = [estp.tile([128, 2, blocklen], bf16, name=f"est{c}",
                                 tag=f"est{c}", bufs=EST_BUFS[c])
                       for c, (_, blocklen) in enumerate(layout)]

        # ---- score entries (the ACT-paced stream) ----
        sc_ready = []
        act_cost = [0.0]
        pe_cost = [0.0]

        mask_pending = {}

        def sc_emit():
            b, c, e = sc_ready.pop(0)
            kt, w, qs, off = layout[c][0][e]
            est = ests[b][c]
            ksl = slice(kt * 128, (kt + 1) * 128)
            ps = spool.tile([128, 2, 512], f32, name="ps", tag="sp")
            nc.tensor.matmul(ps[:, 0, 0:w], kt_[b][0:64, ksl],
                             qt[b][0:64, qs:qs + w], start=True, stop=True)
            nc.tensor.matmul(ps[:, 1, 0:w], kt_[b][64:128, ksl],
                             qt[b][64:128, qs:qs + w], start=True, stop=True)
            nc.scalar.activation(est[:, :, off:off + w], ps[:, :, 0:w],
                                 AF.Exp, scale=0.125)
            if kt >= 4 * c:  # diagonal tile: causal mask, deferred so the
                # exp-gated DVE op doesn't sit ahead of A2A-gating div
                # multiplies in the in-order DVE queue
                mask_pending.setdefault((b, c), []).append(off)
            act_cost[0] += 2 * w * 0.00109 + 0.1

        def emit_masks(b, c):
            est = ests[b][c]
            for off in mask_pending.pop((b, c), []):
                nc.vector.tensor_mul(est[:, :, off:off + 128],
                                     est[:, :, off:off + 128], diagmask2)

        def enq(b, c):
            for e in range(len(layout[c][0])):
                sc_ready.append((b, c, e))

        greedy = [False]

        def pump():
            while sc_ready and (greedy[0]
                                or act_cost[0] < pe_cost[0] + 8.0):
                sc_emit()

        def force(b, c):
            while sc_ready and sc_ready[0][:2] <= (b, c):
                sc_emit()

        def fill(us, fn, *args):
            fn(*args)
            pe_cost[0] += us
            pump()

        # ---- micro-unit worklets ----
        qk_state = {}

        def qk_unit(b, rb, j):
            """Quarter of a q/k projection row-block: chunks 2j, 2j+1."""
            get_batch_tiles(b)
            rsl = slice(rb * 512, (rb + 1) * 512)
            if j == 0:
                qk_state['pqt'] = ppool.tile([128, 512], f32, name="pqt",
                                             tag="pp")
                qk_state['pkt'] = ppool.tile([128, 512], f32, name="pkt",
                                             tag="pp")
            pqt, pkt = qk_state['pqt'], qk_state['pkt']
            for c in (2 * j, 2 * j + 1):
                st = c == 0
                sp = c == NCH - 1
                rhs = xc[:, c, rsl]
                nc.tensor.matmul(pqt, wqT_sb[:, c, :], rhs, start=st, stop=sp)
                nc.tensor.matmul(pkt, wkT_sb[:, c, :], rhs, start=st, stop=sp)
            if j == 3:
                nc.vector.tensor_scalar_add(qt[b][:, rsl], pqt, bq_col)
                nc.vector.tensor_scalar_add(kt_[b][:, rsl], pkt, bk_col)

        def v_unit(b, m0):
            """Two V m-tiles (keys 128*m0 .. 128*m0+256)."""
            for m in (m0, m0 + 1):
                msl = slice(m * 128, (m + 1) * 128)
                pv_ = ppool.tile([128, LD], f32, name="pv", tag="pp")
                for c in range(NCH):
                    nc.tensor.matmul(pv_, xc[:, c, msl], wvT_sb[:, c, :],
                                     start=(c == 0), stop=(c == NCH - 1))
                tv = work.tile([128, LD], f32, name="tv", tag="tv")
                nc.vector.tensor_add(tv, pv_, bv_bc)
                pcol = padf[:, b * NKT + m:b * NKT + m + 1]
                for h in range(2):
                    nc.vector.tensor_scalar_mul(
                        vaug[b][:, h, m, 0:HD], tv[:, h * HD:(h + 1) * HD],
                        pcol)
                    nc.vector.tensor_copy(vaug[b][:, h, m, HD:HD + 1], pcol)

        def pv_mms(b, c, kt_lo, kt_hi, pool=None):
            entries, _ = layout[c]
            est = ests[b][c]
            last = 4 * c + 3
            for h in range(2):
                if kt_lo == 0:
                    po = (pool or pvpool).tile([128, 512], f32,
                                               name=f"po{h}", tag="po" if
                                               pool is None else "pp")
                    pos[(b, c, h)] = po
                po = pos[(b, c, h)]
                for kt, w, qs, off in entries[kt_lo:kt_hi]:
                    po_off = qs - 512 * c
                    nc.tensor.matmul(po[0:HD + 1, po_off:po_off + w],
                                     vaug[b][:, h, kt, :],
                                     est[:, h, off:off + w],
                                     start=(kt == 0), stop=(kt == last))

        def pv_den(b, c):
            for h in range(2):
                po = pos[(b, c, h)]
                den = recp.tile([1, 512], f32, name="den", tag=f"den{h}")
                nc.vector.tensor_copy(den, po[HD:HD + 1, :])
                rec = recp.tile([1, 512], f32, name="rec", tag=f"rec{h}")
                nc.vector.reciprocal_approx_fast(rec, den)
                recb = recp.tile([1, 512], bf16, name="recb",
                                 tag=f"recb{h}", bufs=2)
                nc.vector.tensor_copy(recb, rec)
                recbs[(b, c, h)] = recb

        def pv(b, c):
            pv_mms(b, c, 0, 4 * c + 4)
            pv_den(b, c)

        def div(b, c):
            for h in range(2):
                nc.tensor.matmul(pos[(b, c, h)][64:128, :], ones64,
                                 recbs[(b, c, h)], start=True, stop=True,
                                 skip_group_check=True)
            for h in range(2):
                po = pos[(b, c, h)]
                rbc = recp.tile([HD, 512], bf16, name="rbc", tag=f"rbc{h}")
                nc.vector.tensor_copy(rbc, po[64:128, :])
                nc.vector.tensor_mul(
                    stage[b][h * HD:(h + 1) * HD, 512 * c:512 * (c + 1)],
                    po[0:HD, :], rbc)

        def a2a(b, h2):
            q0, q1 = 1024 * h2, 1024 * (h2 + 1)
            nq = (q1 - q0) // NCORES
            a2a_in = dpool.tile([NCORES * 128, nq], bf16,
                                name=f"a2ai{b}{h2}", tag="a2ai")
            nc.gpsimd.dma_start(
                a2a_in.rearrange("(j p) r -> p j r", p=128),
                stage[b][:, q0:q1].rearrange("p (j r) -> p j r", j=NCORES))
            a2a_out = dpool.tile([NCORES * 128, nq], bf16,
                                 name=f"a2ao{b}{h2}", tag="a2ao")
            nc.gpsimd.collective_compute(
                "AllToAll", ALU.bypass,
                replica_groups=[list(range(NCORES))],
                ins=[a2a_in.opt()], outs=[a2a_out.opt()])
            a2a_outs[(b, h2)] = a2a_out

        op_state = {}

        def op_unit(b, h2, n):
            """Half an output-projection chunk (512 of 1024 out dims)."""
            if n == 0:
                a2a_sb = stg.tile([128, NCORES, 128], bf16,
                                  name=f"a2as{b}{h2}", tag="a2as", bufs=3)
                nc.sync.dma_start(
                    a2a_sb,
                    a2a_outs[(b, h2)].rearrange("(j p) r -> p j r", p=128))
                op_state[(b, h2)] = a2a_sb
            a2a_sb = op_state[(b, h2)]
            r0 = (2 * b + h2) * 128
            pout = ppool.tile([128, 512], f32, name="pout", tag="pp")
            for c in range(NCH):
                nc.tensor.matmul(
                    pout, a2a_sb[:, c, :],
                    woT_sb[:, c, n * 512:(n + 1) * 512],
                    start=(c == 0), stop=(c == NCH - 1))
            ot = work.tile([128, 512], f32, name="ot", tag="ot")
            nc.vector.tensor_add(ot, pout, bo_bc[:, n * 512:(n + 1) * 512])
            nc.sync.dma_start(
                out.ap()[r0:r0 + 128, n * 512:(n + 1) * 512], ot)

        # ---- emission schedule ----
        QKU, VU, OPU, DIVU = 1.05, 1.1, 2.1, 0.55

        # batch-0 projections: V units follow each qk row-block so the
        # xc region's batch-1 reload (issued right behind, Sync queue)
        # unblocks as early as possible
        for rb in range(4):
            for j in range(4):
                fill(QKU, qk_unit, 0, rb, j)
            enq(0, rb)
            fill(VU, v_unit, 0, 4 * rb)
            fill(VU, v_unit, 0, 4 * rb + 2)
            xc_load_rb(1, rb)

        def div_a2a(b, c, h2=None):
            div(b, c)
            if h2 is not None:
                a2a(b, h2)
            pe_cost[0] += DIVU
            pump()

        def pv_stripe(us, b, c):
            force(b, c)
            emit_masks(b, c)
            fill(us, pv, b, c)

        for j in range(4):
            fill(QKU, qk_unit, 1, 0, j)
        enq(1, 0)
        pv_stripe(1.3, 0, 0)
        for j in range(4):
            fill(QKU, qk_unit, 1, 1, j)
        enq(1, 1)
        div_a2a(0, 0)
        pv_stripe(3.4, 0, 1)
        for j in range(4):
            fill(QKU, qk_unit, 1, 2, j)
        enq(1, 2)
        div_a2a(0, 1, 0)
        pv_stripe(5.5, 0, 2)
        for j in range(4):
            fill(QKU, qk_unit, 1, 3, j)
        enq(1, 3)
        div_a2a(0, 2)
        for m0 in range(0, 6, 2):
            fill(VU, v_unit, 1, m0)
        pv_stripe(7.6, 0, 3)
        fill(VU, v_unit, 1, 6)
        div_a2a(0, 3, 1)
        greedy[0] = True
        pump()
        for m0 in range(8, NKT, 2):
            fill(VU, v_unit, 1, m0)
        pv_stripe(1.3, 1, 0)
        div_a2a(1, 0)
        pv_stripe(3.4, 1, 1)
        div_a2a(1, 1, 0)
        pv_stripe(5.5, 1, 2)
        div_a2a(1, 2)
        # stripe 3 of batch 1 is the tail-critical chain: run its
        # non-diagonal PV accumulation (ppool PSUM) as the exp stream
        # drains, then the diagonal tail + normalization right behind
        force(1, 3)
        fill(4.5, pv_mms, 1, 3, 0, 12, ppool)
        emit_masks(1, 3)
        fill(2.0, pv_mms, 1, 3, 12, 16)
        fill(1.1, pv_den, 1, 3)
        div_a2a(1, 3, 1)
        fill(OPU, op_unit, 0, 0, 0)
        fill(OPU, op_unit, 0, 0, 1)
        fill(OPU, op_unit, 0, 1, 0)
        fill(OPU, op_unit, 0, 1, 1)
        fill(OPU, op_unit, 1, 0, 0)
        fill(OPU, op_unit, 1, 0, 1)
        fill(OPU, op_unit, 1, 1, 0)
        fill(OPU, op_unit, 1, 1, 1)

        assert not sc_ready
        assert not mask_pending

    nc.finalize()
    return nc


def _get_nc():
    with _lock:
        if "nc" not in _cache:
            _cache["nc"] = _build_nc()
        return _cache["nc"]


def _shard_inputs(x, input_ids, Wq, bq, Wk, bk, Wv, bv, Wo, bo):
    import ml_dtypes
    bf16 = ml_dtypes.bfloat16

    x = np.asarray(x, dtype=np.float32)
    xT = np.ascontiguousarray(x.reshape(R, D).T).astype(bf16)
    woT = np.ascontiguousarray(np.asarray(Wo, dtype=np.float32).T).astype(bf16)
    bo_f = np.asarray(bo, dtype=np.float32)
    ids = np.asarray(input_ids).astype(np.int32)
    # ids_r[p, b*NKT + t] = input_ids[b, t*128 + p]
    ids_r = np.ascontiguousarray(ids.reshape(B, NKT, 128).transpose(2, 0, 1)
                                 .reshape(128, B * NKT))
    Wq = np.asarray(Wq, dtype=np.float32)
    Wk = np.asarray(Wk, dtype=np.float32)
    Wv = np.asarray(Wv, dtype=np.float32)
    bq = np.asarray(bq, dtype=np.float32)
    bk = np.asarray(bk, dtype=np.float32)
    bv = np.asarray(bv, dtype=np.float32)

    in_maps = []
    for c in range(NCORES):
        sl = slice(c * LD, (c + 1) * LD)
        in_maps.append({
            "xT": xT,
            "wqT": np.ascontiguousarray(Wq[sl].T).astype(bf16),
            "wkT": np.ascontiguousarray(Wk[sl].T).astype(bf16),
            "wvT": np.ascontiguousarray(Wv[sl].T).astype(bf16),
            "woT": woT,
            "bq": bq[sl].copy(),
            "bk": bk[sl].copy(),
            "bv": bv[sl].copy(),
            "bo": bo_f,
            "ids": ids_r,
        })
    return in_maps


def run(trace=False, **inputs):
    """Run the kernel; returns (output, BassKernelResults)."""
    from concourse.bass_utils import run_bass_kernel_spmd

    nc = _get_nc()
    in_maps = _shard_inputs(**inputs)
    res = run_bass_kernel_spmd(nc, in_maps, core_ids=list(range(NCORES)),
                               trace=trace)
    full = np.empty((B, S, D), dtype=np.float32)
    for j in range(NCORES):
        o = np.asarray(res.results[j]["out"], dtype=np.float32)
        for b in range(B):
            for h2 in range(2):
                full[b, 1024 * h2 + 128 * j:1024 * h2 + 128 * (j + 1), :] = \
                    o[(2 * b + h2) * 128:(2 * b + h2 + 1) * 128, :]
    return full, res


def kernel(**inputs) -> np.ndarray:
    full, _ = run(trace=False, **inputs)
    return full


# revision 23
# speedup vs baseline: 1.0851x; 1.0092x over previous
"""Trainium2 Bass kernel for a causal multi-head attention layer.

Model: b=2, s=2048, d_model=1024, 16 heads, head_dim=64, pad-index 0.
Sharding over 8 NeuronCores: each core owns 2 heads (128 of the 1024
attention dims) for both batches (head/tensor parallel).  After attention,
AllToAlls redistribute the per-head outputs so each core holds all 1024
attention dims for 1/8 of the sequence positions, where it runs the output
projection locally.  Output rows per core: 512 (4 chunks of 128).

Schedule: the exp of the attention scores (ACT engine, ~88us for both
batches) is the critical chain, and the PE stream blocks on the 2-deep
score-PSUM ring whenever it runs more than 2 score entries ahead of the
ACT engine.  So the emitter rate-matches: score entries are woven one at
a time between ~1us micro-units of projection / PV / output-projection
work.  A2As run as 4 collectives (one per half-batch) fired as soon as
each half is normalized; a dummy collective at the top absorbs the
collective-stream warmup and core-launch skew.
"""

import threading

import numpy as np

B, S, D = 2, 2048, 1024
H, HD = 16, 64
NCORES = 8
LD = D // NCORES          # 128 local attention dims (2 heads)
R = B * S                 # 4096 flattened rows
RC = R // NCORES          # 512 output rows per core
NKT = S // 128            # 16 key tiles per batch
NCH = D // 128            # 8 contraction chunks of d_model
NST = S // 512            # 4 query stripes per batch

_cache = {}
_lock = threading.Lock()


def _stripe_layout():
    """Per stripe c: list of (kt, width, q_start, offset-in-block), block len."""
    layout = []
    for c in range(NST):
        entries = []
        off = 0
        for kt in range(4 * c + 4):
            qs = max(512 * c, kt * 128)
            w = 512 * (c + 1) - qs
            entries.append((kt, w, qs, off))
            off += w
        layout.append((entries, off))
    return layout


def _build_nc():
    import concourse.mybir as mybir
    import concourse.tile as tile
    from concourse import bacc
    from contextlib import ExitStack

    f32 = mybir.dt.float32
    bf16 = mybir.dt.bfloat16
    i32 = mybir.dt.int32
    AF = mybir.ActivationFunctionType
    ALU = mybir.AluOpType

    nc = bacc.Bacc(None, target_bir_lowering=False, num_devices=NCORES)

    xT = nc.declare_dram_parameter("xT", [D, R], bf16, isOutput=False)
    wqT = nc.declare_dram_parameter("wqT", [D, LD], bf16, isOutput=False)
    wkT = nc.declare_dram_parameter("wkT", [D, LD], bf16, isOutput=False)
    wvT = nc.declare_dram_parameter("wvT", [D, LD], bf16, isOutput=False)
    woT = nc.declare_dram_parameter("woT", [D, D], bf16, isOutput=False)
    bq = nc.declare_dram_parameter("bq", [LD], f32, isOutput=False)
    bk = nc.declare_dram_parameter("bk", [LD], f32, isOutput=False)
    bv = nc.declare_dram_parameter("bv", [LD], f32, isOutput=False)
    bo = nc.declare_dram_parameter("bo", [D], f32, isOutput=False)
    ids = nc.declare_dram_parameter("ids", [128, B * NKT], i32, isOutput=False)
    out = nc.declare_dram_parameter("out", [RC, D], f32, isOutput=True)

    layout = _stripe_layout()

    with ExitStack() as ctx:
        tc = ctx.enter_context(tile.TileContext(nc))
        const = ctx.enter_context(tc.tile_pool(name="const", bufs=1))
        xcp = ctx.enter_context(tc.tile_pool(name="xcp", bufs=1))
        qkp = ctx.enter_context(tc.tile_pool(name="qkp", bufs=2))
        estp = ctx.enter_context(tc.tile_pool(name="estp", bufs=1))
        stg = ctx.enter_context(tc.tile_pool(name="stg", bufs=2))
        work = ctx.enter_context(tc.tile_pool(name="work", bufs=2))
        recp = ctx.enter_context(tc.tile_pool(name="recp", bufs=1))
        ppool = ctx.enter_context(tc.tile_pool(name="ppool", bufs=2, space="PSUM"))
        spool = ctx.enter_context(tc.tile_pool(name="spool", bufs=2, space="PSUM"))
        pvpool = ctx.enter_context(tc.tile_pool(name="pvpool", bufs=2, space="PSUM"))
        dpool = ctx.enter_context(tc.tile_pool(name="dram", bufs=4, space="DRAM"))

        # ---- dummy collective first: syncs the cores and absorbs the
        # collective-stream warmup while the compute phase runs ----
        dummy_i = dpool.tile([8, 16], bf16, name="dummy_i", tag="dummy_i")
        dummy_o = dpool.tile([8, 16], bf16, name="dummy_o", tag="dummy_o")
        nc.gpsimd.collective_compute(
            "AllToAll", ALU.bypass, replica_groups=[list(range(NCORES))],
            ins=[dummy_i.opt()], outs=[dummy_o.opt()])

        # ---- constants on the GpSimd DMA queue so the x loads own Sync ----
        wqT_sb = const.tile([128, NCH, LD], bf16)
        nc.gpsimd.dma_start(wqT_sb, wqT.ap().rearrange("(c p) d -> p c d", p=128))
        wkT_sb = const.tile([128, NCH, LD], bf16)
        nc.gpsimd.dma_start(wkT_sb, wkT.ap().rearrange("(c p) d -> p c d", p=128))
        bq_col = const.tile([128, 1], f32)
        nc.gpsimd.dma_start(bq_col, bq.ap().rearrange("(p o) -> p o", o=1))
        bk_col = const.tile([128, 1], f32)
        nc.gpsimd.dma_start(bk_col, bk.ap().rearrange("(p o) -> p o", o=1))
        wvT_sb = const.tile([128, NCH, LD], bf16)
        nc.gpsimd.dma_start(wvT_sb, wvT.ap().rearrange("(c p) d -> p c d", p=128))
        bv_bc = const.tile([128, LD], f32)
        nc.gpsimd.dma_start(bv_bc, bv.ap().partition_broadcast(128))
        ids_sb = const.tile([128, B * NKT], i32)
        nc.gpsimd.dma_start(ids_sb, ids.ap())
        woT_sb = const.tile([128, NCH, D], bf16)
        nc.gpsimd.dma_start(woT_sb, woT.ap().rearrange("(c p) n -> p c n", p=128))
        bo_bc = const.tile([128, D], f32)
        nc.gpsimd.dma_start(bo_bc, bo.ap().partition_broadcast(128))

        ones64 = const.tile([1, 64], bf16)
        nc.vector.memset(ones64, 1.0)

        # warm-up matmuls: keep the PE HAM busy through the startup DMA
        # window so the first projections run at the full clock
        warm_ps = pvpool.tile([64, 64], f32, name="warm", tag="po")
        for _ in range(90):
            nc.tensor.matmul(warm_ps, ones64, ones64, start=True, stop=True)

        # x^T in one [128, c, r] tile; one big DMA per 512-row block so
        # the Sync queue issues 4 descriptors per batch instead of 32
        xTr = xT.ap().rearrange("(c p) r -> p c r", p=128)
        xc = xcp.tile([128, NCH, S], bf16, name="xc", tag="xc")

        def xc_load_rb(b, rb):
            rsl = slice(rb * 512, (rb + 1) * 512)
            dsl = slice(b * S + rb * 512, b * S + (rb + 1) * 512)
            nc.sync.dma_start(xc[:, :, rsl], xTr[:, :, dsl])

        for rb in range(4):
            xc_load_rb(0, rb)

        padf = const.tile([128, B * NKT], f32)
        nc.vector.tensor_copy(padf, ids_sb)
        nc.vector.tensor_scalar_min(padf, padf, 1.0)

        # diagmask2[x, h, y] = 1 if y >= x else 0 (keys on partitions)
        diagmask = const.tile([128, 128], bf16)
        nc.gpsimd.memset(diagmask, 1.0)
        nc.gpsimd.affine_select(
            out=diagmask, in_=diagmask, compare_op=ALU.is_ge, fill=0.0,
            base=0, pattern=[[1, 128]], channel_multiplier=-1,
        )
        diagmask2 = const.tile([128, 2, 128], bf16)
        nc.vector.tensor_copy(diagmask2[:, 0, :], diagmask)
        nc.vector.tensor_copy(diagmask2[:, 1, :], diagmask)

        # ---- per-batch persistent tiles ----
        qt = {}
        kt_ = {}
        vaug = {}
        stage = {}
        ests = {}
        pos = {}
        recbs = {}
        a2a_outs = {}

        EST_BUFS = [2, 2, 1, 1]

        def get_batch_tiles(b):
            if b in qt:
                return
            qt[b] = qkp.tile([128, S], bf16, name=f"qt{b}", tag="qt")
            kt_[b] = qkp.tile([128, S], bf16, name=f"kt{b}", tag="kt")
            vaug[b] = qkp.tile([128, 2, NKT, HD + 1], bf16, name=f"vaug{b}",
                               tag="vaug")
            stage[b] = stg.tile([128, S], bf16, name=f"stage{b}", tag="stage")
            ests[b] = [estp.tile([128, 2, blocklen], bf16, name=f"est{c}",
                                 tag=f"est{c}", bufs=EST_BUFS[c])
                       for c, (_, blocklen) in enumerate(layout)]

        # ---- score entries (the ACT-paced stream) ----
        sc_ready = []
        act_cost = [0.0]
        pe_cost = [0.0]

        mask_pending = {}

        def sc_emit():
            b, c, e = sc_ready.pop(0)
            kt, w, qs, off = layout[c][0][e]
            est = ests[b][c]
            ksl = slice(kt * 128, (kt + 1) * 128)
            ps = spool.tile([128, 2, 512], f32, name="ps", tag="sp")
            nc.tensor.matmul(ps[:, 0, 0:w], kt_[b][0:64, ksl],
                             qt[b][0:64, qs:qs + w], start=True, stop=True)
            nc.tensor.matmul(ps[:, 1, 0:w], kt_[b][64:128, ksl],
                             qt[b][64:128, qs:qs + w], start=True, stop=True)
            nc.scalar.activation(est[:, :, off:off + w], ps[:, :, 0:w],
                                 AF.Exp, scale=0.125)
            if kt >= 4 * c:  # diagonal tile: causal mask, deferred so the
                # exp-gated DVE op doesn't sit ahead of A2A-gating div
                # multiplies in the in-order DVE queue
                mask_pending.setdefault((b, c), []).append(off)
            act_cost[0] += 2 * w * 0.00109 + 0.1

        def emit_masks(b, c):
            est = ests[b][c]
            for off in mask_pending.pop((b, c), []):
                nc.vector.tensor_mul(est[:, :, off:off + 128],
                                     est[:, :, off:off + 128], diagmask2)

        def enq(b, c):
            for e in range(len(layout[c][0])):
                sc_ready.append((b, c, e))

        greedy = [False]

        def pump():
            while sc_ready and (greedy[0]
                                or act_cost[0] < pe_cost[0] + 8.0):
                sc_emit()

        def force(b, c):
            while sc_ready and sc_ready[0][:2] <= (b, c):
                sc_emit()

        def fill(us, fn, *args):
            fn(*args)
            pe_cost[0] += us
            pump()

        # ---- micro-unit worklets ----
        qk_state = {}

        def qk_unit(b, rb, j):
            """Quarter of a q/k projection row-block: chunks 2j, 2j+1."""
            get_batch_tiles(b)
            rsl = slice(rb * 512, (rb + 1) * 512)
            if j == 0:
                qk_state['pqt'] = ppool.tile([128, 512], f32, name="pqt",
                                             tag="pp")
                qk_state['pkt'] = ppool.tile([128, 512], f32, name="pkt",
                                             tag="pp")
            pqt, pkt = qk_state['pqt'], qk_state['pkt']
            for c in (2 * j, 2 * j + 1):
                st = c == 0
                sp = c == NCH - 1
                rhs = xc[:, c, rsl]
                nc.tensor.matmul(pqt, wqT_sb[:, c, :], rhs, start=st, stop=sp)
                nc.tensor.matmul(pkt, wkT_sb[:, c, :], rhs, start=st, stop=sp)
            if j == 3:
                nc.vector.tensor_scalar_add(qt[b][:, rsl], pqt, bq_col)
                nc.vector.tensor_scalar_add(kt_[b][:, rsl], pkt, bk_col)

        def v_unit(b, m0):
            """Two V m-tiles (keys 128*m0 .. 128*m0+256)."""
            for m in (m0, m0 + 1):
                msl = slice(m * 128, (m + 1) * 128)
                pv_ = ppool.tile([128, LD], f32, name="pv", tag="pp")
                for c in range(NCH):
                    nc.tensor.matmul(pv_, xc[:, c, msl], wvT_sb[:, c, :],
                                     start=(c == 0), stop=(c == NCH - 1))
                tv = work.tile([128, LD], f32, name="tv", tag="tv")
                nc.vector.tensor_add(tv, pv_, bv_bc)
                pcol = padf[:, b * NKT + m:b * NKT + m + 1]
                for h in range(2):
                    nc.vector.tensor_scalar_mul(
                        vaug[b][:, h, m, 0:HD], tv[:, h * HD:(h + 1) * HD],
                        pcol)
                    nc.vector.tensor_copy(vaug[b][:, h, m, HD:HD + 1], pcol)

        def pv_mms(b, c, kt_lo, kt_hi, pool=None):
            entries, _ = layout[c]
            est = ests[b][c]
            last = 4 * c + 3
            for h in range(2):
                if kt_lo == 0:
                    po = (pool or pvpool).tile([128, 512], f32,
                                               name=f"po{h}", tag="po" if
                                               pool is None else "pp")
                    pos[(b, c, h)] = po
                po = pos[(b, c, h)]
                for kt, w, qs, off in entries[kt_lo:kt_hi]:
                    po_off = qs - 512 * c
                    nc.tensor.matmul(po[0:HD + 1, po_off:po_off + w],
                                     vaug[b][:, h, kt, :],
                                     est[:, h, off:off + w],
                                     start=(kt == 0), stop=(kt == last))

        def pv_den(b, c):
            for h in range(2):
                po = pos[(b, c, h)]
                den = recp.tile([1, 512], f32, name="den", tag=f"den{h}")
                nc.vector.tensor_copy(den, po[HD:HD + 1, :])
                rec = recp.tile([1, 512], f32, name="rec", tag=f"rec{h}")
                nc.vector.reciprocal_approx_fast(rec, den)
                recb = recp.tile([1, 512], bf16, name="recb",
                                 tag=f"recb{h}", bufs=2)
                nc.vector.tensor_copy(recb, rec)
                recbs[(b, c, h)] = recb

        def pv(b, c):
            pv_mms(b, c, 0, 4 * c + 4)
            pv_den(b, c)

        def div(b, c):
            for h in range(2):
                nc.tensor.matmul(pos[(b, c, h)][64:128, :], ones64,
                                 recbs[(b, c, h)], start=True, stop=True,
                                 skip_group_check=True)
            for h in range(2):
                po = pos[(b, c, h)]
                rbc = recp.tile([HD, 512], bf16, name="rbc", tag=f"rbc{h}")
                nc.vector.tensor_copy(rbc, po[64:128, :])
                nc.vector.tensor_mul(
                    stage[b][h * HD:(h + 1) * HD, 512 * c:512 * (c + 1)],
                    po[0:HD, :], rbc)

        def a2a(key, b, q0, q1):
            nq = (q1 - q0) // NCORES
            a2a_in = dpool.tile([NCORES * 128, nq], bf16,
                                name=f"a2ai{key}", tag="a2ai")
            nc.gpsimd.dma_start(
                a2a_in.rearrange("(j p) r -> p j r", p=128),
                stage[b][:, q0:q1].rearrange("p (j r) -> p j r", j=NCORES))
            a2a_out = dpool.tile([NCORES * 128, nq], bf16,
                                 name=f"a2ao{key}", tag="a2ao")
            nc.gpsimd.collective_compute(
                "AllToAll", ALU.bypass,
                replica_groups=[list(range(NCORES))],
                ins=[a2a_in.opt()], outs=[a2a_out.opt()])
            a2a_outs[key] = (a2a_out, nq)

        op_state = {}

        def op_unit(key, rc, n, r0):
            """One 128-row x 512-outdim piece of the output projection."""
            a2a_out, nq = a2a_outs[key]
            if (rc, n) == (0, 0):
                a2a_sb = stg.tile([128, NCORES, nq], bf16,
                                  name=f"a2as{key}", tag="a2as", bufs=2)
                nc.sync.dma_start(
                    a2a_sb, a2a_out.rearrange("(j p) r -> p j r", p=128))
                op_state[key] = a2a_sb
            a2a_sb = op_state[key]
            pout = ppool.tile([128, 512], f32, name="pout", tag="pp")
            for c in range(NCH):
                nc.tensor.matmul(
                    pout, a2a_sb[:, c, rc * 128:(rc + 1) * 128],
                    woT_sb[:, c, n * 512:(n + 1) * 512],
                    start=(c == 0), stop=(c == NCH - 1))
            ot = work.tile([128, 512], f32, name="ot", tag="ot")
            nc.vector.tensor_add(ot, pout, bo_bc[:, n * 512:(n + 1) * 512])
            nc.sync.dma_start(
                out.ap()[r0 + rc * 128:r0 + (rc + 1) * 128,
                         n * 512:(n + 1) * 512], ot)

        # ---- emission schedule ----
        QKU, VU, OPU, DIVU = 1.05, 1.1, 2.1, 0.55

        # batch-0 projections: V units follow each qk row-block so the
        # xc region's batch-1 reload (issued right behind, Sync queue)
        # unblocks as early as possible
        for rb in range(4):
            for j in range(4):
                fill(QKU, qk_unit, 0, rb, j)
            enq(0, rb)
            fill(VU, v_unit, 0, 4 * rb)
            fill(VU, v_unit, 0, 4 * rb + 2)
            xc_load_rb(1, rb)

        def div_a2a(b, c, a2a_args=None):
            div(b, c)
            if a2a_args is not None:
                a2a(*a2a_args)
            pe_cost[0] += DIVU
            pump()

        def pv_stripe(us, b, c):
            force(b, c)
            emit_masks(b, c)
            fill(us, pv, b, c)

        for j in range(4):
            fill(QKU, qk_unit, 1, 0, j)
        enq(1, 0)
        pv_stripe(1.3, 0, 0)
        for j in range(4):
            fill(QKU, qk_unit, 1, 1, j)
        enq(1, 1)
        div_a2a(0, 0)
        pv_stripe(3.4, 0, 1)
        for j in range(4):
            fill(QKU, qk_unit, 1, 2, j)
        enq(1, 2)
        div_a2a(0, 1)
        pv_stripe(5.5, 0, 2)
        for j in range(4):
            fill(QKU, qk_unit, 1, 3, j)
        enq(1, 3)
        div_a2a(0, 2)
        for m0 in range(0, 6, 2):
            fill(VU, v_unit, 1, m0)
        pv_stripe(7.6, 0, 3)
        fill(VU, v_unit, 1, 6)
        div_a2a(0, 3, ("b0", 0, 0, S))
        for m0 in range(8, 12, 2):
            fill(VU, v_unit, 1, m0)
        pv_stripe(1.3, 1, 0)
        div_a2a(1, 0)
        pv_stripe(3.4, 1, 1)
        div_a2a(1, 1, ("b1a", 1, 0, 1024))
        for m0 in range(12, NKT, 2):
            fill(VU, v_unit, 1, m0)
        pv_stripe(5.5, 1, 2)
        div_a2a(1, 2)
        greedy[0] = True
        pump()
        # stripe 3 of batch 1 is the tail-critical chain: run its
        # non-diagonal PV accumulation (ppool PSUM) as the exp stream
        # drains, then the diagonal tail + normalization right behind
        force(1, 3)
        fill(4.5, pv_mms, 1, 3, 0, 12, ppool)
        emit_masks(1, 3)
        fill(2.0, pv_mms, 1, 3, 12, 16)
        fill(1.1, pv_den, 1, 3)
        div_a2a(1, 3, ("b1b", 1, 1024, S))
        for rc in range(2):
            for n in range(2):
                fill(OPU, op_unit, "b0", rc, n, 0)
        for n in range(2):
            fill(OPU, op_unit, "b1a", 0, n, 256)
        for n in range(2):
            fill(OPU, op_unit, "b1b", 0, n, 384)

        assert not sc_ready
        assert not mask_pending

    nc.finalize()
    return nc


def _get_nc():
    with _lock:
        if "nc" not in _cache:
            _cache["nc"] = _build_nc()
        return _cache["nc"]


def _shard_inputs(x, input_ids, Wq, bq, Wk, bk, Wv, bv, Wo, bo):
    import ml_dtypes
    bf16 = ml_dtypes.bfloat16

    x = np.asarray(x, dtype=np.float32)
    xT = np.ascontiguousarray(x.reshape(R, D).T).astype(bf16)
    woT = np.ascontiguousarray(np.asarray(Wo, dtype=np.float32).T).astype(bf16)
    bo_f = np.asarray(bo, dtype=np.float32)
    ids = np.asarray(input_ids).astype(np.int32)
    # ids_r[p, b*NKT + t] = input_ids[b, t*128 + p]
    ids_r = np.ascontiguousarray(ids.reshape(B, NKT, 128).transpose(2, 0, 1)
                                 .reshape(128, B * NKT))
    Wq = np.asarray(Wq, dtype=np.float32)
    Wk = np.asarray(Wk, dtype=np.float32)
    Wv = np.asarray(Wv, dtype=np.float32)
    bq = np.asarray(bq, dtype=np.float32)
    bk = np.asarray(bk, dtype=np.float32)
    bv = np.asarray(bv, dtype=np.float32)

    in_maps = []
    for c in range(NCORES):
        sl = slice(c * LD, (c + 1) * LD)
        in_maps.append({
            "xT": xT,
            "wqT": np.ascontiguousarray(Wq[sl].T).astype(bf16),
            "wkT": np.ascontiguousarray(Wk[sl].T).astype(bf16),
            "wvT": np.ascontiguousarray(Wv[sl].T).astype(bf16),
            "woT": woT,
            "bq": bq[sl].copy(),
            "bk": bk[sl].copy(),
            "bv": bv[sl].copy(),
            "bo": bo_f,
            "ids": ids_r,
        })
    return in_maps


def run(trace=False, **inputs):
    """Run the kernel; returns (output, BassKernelResults)."""
    from concourse.bass_utils import run_bass_kernel_spmd

    nc = _get_nc()
    in_maps = _shard_inputs(**inputs)
    res = run_bass_kernel_spmd(nc, in_maps, core_ids=list(range(NCORES)),
                               trace=trace)
    full = np.empty((B, S, D), dtype=np.float32)
    for j in range(NCORES):
        o = np.asarray(res.results[j]["out"], dtype=np.float32)
        # batch 0: one A2A, core j owns 256 contiguous queries
        full[0, 256 * j:256 * (j + 1), :] = o[0:256, :]
        # batch 1: two A2A halves, 128 queries per core each
        for h2 in range(2):
            full[1, 1024 * h2 + 128 * j:1024 * h2 + 128 * (j + 1), :] = \
                o[256 + h2 * 128:256 + (h2 + 1) * 128, :]
    return full, res


def kernel(**inputs) -> np.ndarray:
    full, _ = run(trace=False, **inputs)
    return full


# revision 27
# speedup vs baseline: 1.0895x; 1.0040x over previous
"""Trainium2 Bass kernel for a causal multi-head attention layer.

Model: b=2, s=2048, d_model=1024, 16 heads, head_dim=64, pad-index 0.
Sharding over 8 NeuronCores: each core owns 2 heads (128 of the 1024
attention dims) for both batches (head/tensor parallel).  After attention,
AllToAlls redistribute the per-head outputs so each core holds all 1024
attention dims for 1/8 of the sequence positions, where it runs the output
projection locally.  Output rows per core: 512 (4 chunks of 128).

Schedule: the exp of the attention scores (ACT engine, ~88us for both
batches) is the critical chain, and the PE stream blocks on the 2-deep
score-PSUM ring whenever it runs more than 2 score entries ahead of the
ACT engine.  So the emitter rate-matches: score entries are woven one at
a time between ~1us micro-units of projection / PV / output-projection
work.  A2As run as 4 collectives (one per half-batch) fired as soon as
each half is normalized; a dummy collective at the top absorbs the
collective-stream warmup and core-launch skew.
"""

import threading

import numpy as np

B, S, D = 2, 2048, 1024
H, HD = 16, 64
NCORES = 8
LD = D // NCORES          # 128 local attention dims (2 heads)
R = B * S                 # 4096 flattened rows
RC = R // NCORES          # 512 output rows per core
NKT = S // 128            # 16 key tiles per batch
NCH = D // 128            # 8 contraction chunks of d_model
NST = S // 512            # 4 query stripes per batch

_cache = {}
_lock = threading.Lock()


def _stripe_layout():
    """Per stripe c: list of (kt, width, q_start, offset-in-block), block len."""
    layout = []
    for c in range(NST):
        entries = []
        off = 0
        for kt in range(4 * c + 4):
            qs = max(512 * c, kt * 128)
            w = 512 * (c + 1) - qs
            entries.append((kt, w, qs, off))
            off += w
        layout.append((entries, off))
    return layout


def _build_nc():
    import concourse.mybir as mybir
    import concourse.tile as tile
    from concourse import bacc
    from contextlib import ExitStack

    f32 = mybir.dt.float32
    bf16 = mybir.dt.bfloat16
    i32 = mybir.dt.int32
    AF = mybir.ActivationFunctionType
    ALU = mybir.AluOpType

    nc = bacc.Bacc(None, target_bir_lowering=False, num_devices=NCORES)

    xT = nc.declare_dram_parameter("xT", [D, R], bf16, isOutput=False)
    wqT = nc.declare_dram_parameter("wqT", [D, LD], bf16, isOutput=False)
    wkT = nc.declare_dram_parameter("wkT", [D, LD], bf16, isOutput=False)
    wvT = nc.declare_dram_parameter("wvT", [D, LD], bf16, isOutput=False)
    woT = nc.declare_dram_parameter("woT", [D, D], bf16, isOutput=False)
    bq = nc.declare_dram_parameter("bq", [LD], f32, isOutput=False)
    bk = nc.declare_dram_parameter("bk", [LD], f32, isOutput=False)
    bv = nc.declare_dram_parameter("bv", [LD], f32, isOutput=False)
    bo = nc.declare_dram_parameter("bo", [D], f32, isOutput=False)
    ids = nc.declare_dram_parameter("ids", [128, B * NKT], i32, isOutput=False)
    out = nc.declare_dram_parameter("out", [RC, D], f32, isOutput=True)

    layout = _stripe_layout()

    with ExitStack() as ctx:
        tc = ctx.enter_context(tile.TileContext(nc))
        const = ctx.enter_context(tc.tile_pool(name="const", bufs=1))
        xcp = ctx.enter_context(tc.tile_pool(name="xcp", bufs=1))
        qkp = ctx.enter_context(tc.tile_pool(name="qkp", bufs=2))
        estp = ctx.enter_context(tc.tile_pool(name="estp", bufs=1))
        stg = ctx.enter_context(tc.tile_pool(name="stg", bufs=2))
        work = ctx.enter_context(tc.tile_pool(name="work", bufs=2))
        recp = ctx.enter_context(tc.tile_pool(name="recp", bufs=1))
        ppool = ctx.enter_context(tc.tile_pool(name="ppool", bufs=2, space="PSUM"))
        spool = ctx.enter_context(tc.tile_pool(name="spool", bufs=2, space="PSUM"))
        pvpool = ctx.enter_context(tc.tile_pool(name="pvpool", bufs=2, space="PSUM"))
        dpool = ctx.enter_context(tc.tile_pool(name="dram", bufs=4, space="DRAM"))

        # ---- dummy collective first: syncs the cores and absorbs the
        # collective-stream warmup while the compute phase runs ----
        dummy_i = dpool.tile([8, 16], bf16, name="dummy_i", tag="dummy_i")
        dummy_o = dpool.tile([8, 16], bf16, name="dummy_o", tag="dummy_o")
        nc.gpsimd.collective_compute(
            "AllToAll", ALU.bypass, replica_groups=[list(range(NCORES))],
            ins=[dummy_i.opt()], outs=[dummy_o.opt()])

        # ---- constants on the GpSimd DMA queue so the x loads own Sync ----
        wqT_sb = const.tile([128, NCH, LD], bf16)
        nc.gpsimd.dma_start(wqT_sb, wqT.ap().rearrange("(c p) d -> p c d", p=128))
        wkT_sb = const.tile([128, NCH, LD], bf16)
        nc.gpsimd.dma_start(wkT_sb, wkT.ap().rearrange("(c p) d -> p c d", p=128))
        bq_col = const.tile([128, 1], f32)
        nc.gpsimd.dma_start(bq_col, bq.ap().rearrange("(p o) -> p o", o=1))
        bk_col = const.tile([128, 1], f32)
        nc.gpsimd.dma_start(bk_col, bk.ap().rearrange("(p o) -> p o", o=1))
        wvT_sb = const.tile([128, NCH, LD], bf16)
        nc.gpsimd.dma_start(wvT_sb, wvT.ap().rearrange("(c p) d -> p c d", p=128))
        bv_bc = const.tile([128, LD], f32)
        nc.gpsimd.dma_start(bv_bc, bv.ap().partition_broadcast(128))
        ids_sb = const.tile([128, B * NKT], i32)
        nc.gpsimd.dma_start(ids_sb, ids.ap())
        woT_sb = const.tile([128, NCH, D], bf16)
        nc.gpsimd.dma_start(woT_sb, woT.ap().rearrange("(c p) n -> p c n", p=128))
        bo_bc = const.tile([128, D], f32)
        nc.gpsimd.dma_start(bo_bc, bo.ap().partition_broadcast(128))

        ones64 = const.tile([1, 64], bf16)
        nc.vector.memset(ones64, 1.0)

        # warm-up matmuls: keep the PE HAM busy through the startup DMA
        # window so the first projections run at the full clock
        warm_ps = pvpool.tile([64, 64], f32, name="warm", tag="po")
        for _ in range(90):
            nc.tensor.matmul(warm_ps, ones64, ones64, start=True, stop=True)

        # x^T in one [128, c, r] tile; one big DMA per 512-row block so
        # the Sync queue issues 4 descriptors per batch instead of 32
        xTr = xT.ap().rearrange("(c p) r -> p c r", p=128)
        xc = xcp.tile([128, NCH, S], bf16, name="xc", tag="xc")

        def xc_load_rb(b, rb):
            rsl = slice(rb * 512, (rb + 1) * 512)
            dsl = slice(b * S + rb * 512, b * S + (rb + 1) * 512)
            nc.sync.dma_start(xc[:, :, rsl], xTr[:, :, dsl])

        for rb in range(4):
            xc_load_rb(0, rb)

        padf = const.tile([128, B * NKT], f32)
        nc.vector.tensor_copy(padf, ids_sb)
        nc.vector.tensor_scalar_min(padf, padf, 1.0)

        # diagmask2[x, h, y] = 1 if y >= x else 0 (keys on partitions)
        diagmask = const.tile([128, 128], bf16)
        nc.gpsimd.memset(diagmask, 1.0)
        nc.gpsimd.affine_select(
            out=diagmask, in_=diagmask, compare_op=ALU.is_ge, fill=0.0,
            base=0, pattern=[[1, 128]], channel_multiplier=-1,
        )
        diagmask2 = const.tile([128, 2, 128], bf16)
        nc.vector.tensor_copy(diagmask2[:, 0, :], diagmask)
        nc.vector.tensor_copy(diagmask2[:, 1, :], diagmask)

        # ---- per-batch persistent tiles ----
        qt = {}
        kt_ = {}
        vaug = {}
        stage = {}
        ests = {}
        pos = {}
        recbs = {}
        a2a_outs = {}

        EST_BUFS = [2, 2, 1, 1]

        def get_batch_tiles(b):
            if b in qt:
                return
            qt[b] = qkp.tile([128, S], bf16, name=f"qt{b}", tag="qt")
            kt_[b] = qkp.tile([128, S], bf16, name=f"kt{b}", tag="kt")
            vaug[b] = qkp.tile([128, 2, NKT, HD + 1], bf16, name=f"vaug{b}",
                               tag="vaug")
            stage[b] = stg.tile([128, S], bf16, name=f"stage{b}", tag="stage")
            ests[b] = [estp.tile([128, 2, blocklen], bf16, name=f"est{c}",
                                 tag=f"est{c}", bufs=EST_BUFS[c])
                       for c, (_, blocklen) in enumerate(layout)]

        # ---- score entries (the ACT-paced stream) ----
        sc_ready = []
        act_cost = [0.0]
        pe_cost = [0.0]

        mask_pending = {}

        def sc_emit():
            b, c, e = sc_ready.pop(0)
            kt, w, qs, off = layout[c][0][e]
            est = ests[b][c]
            ksl = slice(kt * 128, (kt + 1) * 128)
            ps = spool.tile([128, 2, 512], f32, name="ps", tag="sp")
            nc.tensor.matmul(ps[:, 0, 0:w], kt_[b][0:64, ksl],
                             qt[b][0:64, qs:qs + w], start=True, stop=True)
            nc.tensor.matmul(ps[:, 1, 0:w], kt_[b][64:128, ksl],
                             qt[b][64:128, qs:qs + w], start=True, stop=True)
            nc.scalar.activation(est[:, :, off:off + w], ps[:, :, 0:w],
                                 AF.Exp, scale=0.125)
            if kt >= 4 * c:  # diagonal tile: causal mask, deferred so the
                # exp-gated DVE op doesn't sit ahead of A2A-gating div
                # multiplies in the in-order DVE queue
                mask_pending.setdefault((b, c), []).append(off)
            act_cost[0] += 2 * w * 0.00109 + 0.1

        def emit_masks(b, c):
            est = ests[b][c]
            for off in mask_pending.pop((b, c), []):
                nc.vector.tensor_mul(est[:, :, off:off + 128],
                                     est[:, :, off:off + 128], diagmask2)

        def enq(b, c):
            for e in range(len(layout[c][0])):
                sc_ready.append((b, c, e))

        greedy = [False]

        n_emitted = [0]

        def pump():
            # An entry's score matmuls block the PE (2-deep PSUM ring)
            # until the exp two entries back has drained, so entries must
            # land in the PE stream just-in-time: ACT position trails the
            # PE position by the engines' start offset (~9us) minus ring.
            while sc_ready and (greedy[0] or n_emitted[0] < 3
                                or act_cost[0] < pe_cost[0] - 5.0):
                sc_emit()
                n_emitted[0] += 1

        def force(b, c):
            while sc_ready and sc_ready[0][:2] <= (b, c):
                sc_emit()

        def fill(us, fn, *args):
            fn(*args)
            pe_cost[0] += us
            pump()

        # ---- micro-unit worklets ----
        qk_state = {}

        def qk_unit(b, rb, j):
            """Quarter of a q/k projection row-block: chunks 2j, 2j+1."""
            get_batch_tiles(b)
            rsl = slice(rb * 512, (rb + 1) * 512)
            if j == 0:
                qk_state['pqt'] = ppool.tile([128, 512], f32, name="pqt",
                                             tag="pp")
                qk_state['pkt'] = ppool.tile([128, 512], f32, name="pkt",
                                             tag="pp")
            pqt, pkt = qk_state['pqt'], qk_state['pkt']
            for c in (2 * j, 2 * j + 1):
                st = c == 0
                sp = c == NCH - 1
                rhs = xc[:, c, rsl]
                nc.tensor.matmul(pqt, wqT_sb[:, c, :], rhs, start=st, stop=sp)
                nc.tensor.matmul(pkt, wkT_sb[:, c, :], rhs, start=st, stop=sp)
            if j == 3:
                nc.vector.tensor_scalar_add(qt[b][:, rsl], pqt, bq_col)
                nc.vector.tensor_scalar_add(kt_[b][:, rsl], pkt, bk_col)

        def v_unit(b, m0):
            """Two V m-tiles (keys 128*m0 .. 128*m0+256)."""
            for m in (m0, m0 + 1):
                msl = slice(m * 128, (m + 1) * 128)
                pv_ = ppool.tile([128, LD], f32, name="pv", tag="pp")
                for c in range(NCH):
                    nc.tensor.matmul(pv_, xc[:, c, msl], wvT_sb[:, c, :],
                                     start=(c == 0), stop=(c == NCH - 1))
                tv = work.tile([128, LD], f32, name="tv", tag="tv")
                nc.vector.tensor_add(tv, pv_, bv_bc)
                pcol = padf[:, b * NKT + m:b * NKT + m + 1]
                for h in range(2):
                    nc.vector.tensor_scalar_mul(
                        vaug[b][:, h, m, 0:HD], tv[:, h * HD:(h + 1) * HD],
                        pcol)
                    nc.vector.tensor_copy(vaug[b][:, h, m, HD:HD + 1], pcol)

        def pv_mms(b, c, kt_lo, kt_hi, pool=None):
            entries, _ = layout[c]
            est = ests[b][c]
            last = 4 * c + 3
            for h in range(2):
                if kt_lo == 0:
                    po = (pool or pvpool).tile([128, 512], f32,
                                               name=f"po{h}", tag="po" if
                                               pool is None else "pp")
                    pos[(b, c, h)] = po
                po = pos[(b, c, h)]
                for kt, w, qs, off in entries[kt_lo:kt_hi]:
                    po_off = qs - 512 * c
                    nc.tensor.matmul(po[0:HD + 1, po_off:po_off + w],
                                     vaug[b][:, h, kt, :],
                                     est[:, h, off:off + w],
                                     start=(kt == 0), stop=(kt == last))

        def pv_den(b, c):
            for h in range(2):
                po = pos[(b, c, h)]
                den = recp.tile([1, 512], f32, name="den", tag=f"den{h}")
                nc.vector.tensor_copy(den, po[HD:HD + 1, :])
                rec = recp.tile([1, 512], f32, name="rec", tag=f"rec{h}")
                nc.vector.reciprocal_approx_fast(rec, den)
                recb = recp.tile([1, 512], bf16, name="recb",
                                 tag=f"recb{h}", bufs=2)
                nc.vector.tensor_copy(recb, rec)
                recbs[(b, c, h)] = recb

        def pv(b, c):
            pv_mms(b, c, 0, 4 * c + 4)
            pv_den(b, c)

        def div(b, c):
            for h in range(2):
                nc.tensor.matmul(pos[(b, c, h)][64:128, :], ones64,
                                 recbs[(b, c, h)], start=True, stop=True,
                                 skip_group_check=True)
            for h in range(2):
                po = pos[(b, c, h)]
                rbc = recp.tile([HD, 512], bf16, name="rbc", tag=f"rbc{h}")
                nc.vector.tensor_copy(rbc, po[64:128, :])
                nc.vector.tensor_mul(
                    stage[b][h * HD:(h + 1) * HD, 512 * c:512 * (c + 1)],
                    po[0:HD, :], rbc)

        def a2a(key, b, q0, q1):
            nq = (q1 - q0) // NCORES
            a2a_in = dpool.tile([NCORES * 128, nq], bf16,
                                name=f"a2ai{key}", tag="a2ai")
            nc.gpsimd.dma_start(
                a2a_in.rearrange("(j p) r -> p j r", p=128),
                stage[b][:, q0:q1].rearrange("p (j r) -> p j r", j=NCORES))
            a2a_out = dpool.tile([NCORES * 128, nq], bf16,
                                 name=f"a2ao{key}", tag="a2ao")
            nc.gpsimd.collective_compute(
                "AllToAll", ALU.bypass,
                replica_groups=[list(range(NCORES))],
                ins=[a2a_in.opt()], outs=[a2a_out.opt()])
            a2a_outs[key] = (a2a_out, nq)

        op_state = {}

        def op_unit(key, rc, n, r0):
            """One 128-row x 512-outdim piece of the output projection."""
            a2a_out, nq = a2a_outs[key]
            if (rc, n) == (0, 0):
                a2a_sb = stg.tile([128, NCORES, nq], bf16,
                                  name=f"a2as{key}", tag="a2as", bufs=2)
                nc.sync.dma_start(
                    a2a_sb, a2a_out.rearrange("(j p) r -> p j r", p=128))
                op_state[key] = a2a_sb
            a2a_sb = op_state[key]
            pout = ppool.tile([128, 512], f32, name="pout", tag="pp")
            for c in range(NCH):
                nc.tensor.matmul(
                    pout, a2a_sb[:, c, rc * 128:(rc + 1) * 128],
                    woT_sb[:, c, n * 512:(n + 1) * 512],
                    start=(c == 0), stop=(c == NCH - 1))
            ot = work.tile([128, 512], f32, name="ot", tag="ot")
            nc.vector.tensor_add(ot, pout, bo_bc[:, n * 512:(n + 1) * 512])
            nc.sync.dma_start(
                out.ap()[r0 + rc * 128:r0 + (rc + 1) * 128,
                         n * 512:(n + 1) * 512], ot)

        # ---- emission schedule ----
        QKU, VU, OPU, DIVU = 1.05, 1.1, 2.1, 0.55

        # batch-0 projections: V units follow each qk row-block so the
        # xc region's batch-1 reload (issued right behind, Sync queue)
        # unblocks as early as possible
        for rb in range(4):
            for j in range(4):
                fill(QKU, qk_unit, 0, rb, j)
            enq(0, rb)
            fill(VU, v_unit, 0, 4 * rb)
            fill(VU, v_unit, 0, 4 * rb + 2)
            xc_load_rb(1, rb)

        def div_a2a(b, c, a2a_args=None):
            div(b, c)
            if a2a_args is not None:
                a2a(*a2a_args)
            pe_cost[0] += DIVU
            pump()

        def pv_stripe(b, c, pool=None):
            force(b, c)
            entries = layout[c][0]
            last = len(entries)
            for lo in range(0, last, 4):
                hi = min(lo + 4, last)
                if hi == last:  # chunk containing the diagonal tiles
                    emit_masks(b, c)
                us = sum(2 * entries[e][1] for e in range(lo, hi)) / 1950.0
                fill(us, pv_mms, b, c, lo, hi, pool)
            fill(0.6, pv_den, b, c)

        for j in range(4):
            fill(QKU, qk_unit, 1, 0, j)
        enq(1, 0)
        pv_stripe(0, 0)
        for j in range(4):
            fill(QKU, qk_unit, 1, 1, j)
        enq(1, 1)
        div_a2a(0, 0)
        pv_stripe(0, 1)
        for j in range(4):
            fill(QKU, qk_unit, 1, 2, j)
        div_a2a(0, 1)
        pv_stripe(0, 2)
        for j in range(4):
            fill(QKU, qk_unit, 1, 3, j)
        div_a2a(0, 2)
        for m0 in range(0, 6, 2):
            fill(VU, v_unit, 1, m0)
        pv_stripe(0, 3)
        fill(VU, v_unit, 1, 6)
        div_a2a(0, 3, ("b0", 0, 0, S))
        enq(1, 2)
        for m0 in range(8, 12, 2):
            fill(VU, v_unit, 1, m0)
        pv_stripe(1, 0)
        div_a2a(1, 0)
        pv_stripe(1, 1)
        div_a2a(1, 1, ("b1a", 1, 0, 1024))
        enq(1, 3)
        for m0 in range(12, NKT, 2):
            fill(VU, v_unit, 1, m0)
        pv_stripe(1, 2)
        div_a2a(1, 2)
        greedy[0] = True
        pump()
        # stripe 3 of batch 1 is the tail-critical chain: its PV runs in
        # the ppool PSUM so it can accumulate while pv(1,2) still awaits
        # its division readout
        pv_stripe(1, 3, ppool)
        div_a2a(1, 3, ("b1b", 1, 1024, S))
        for rc in range(2):
            for n in range(2):
                fill(OPU, op_unit, "b0", rc, n, 0)
        for n in range(2):
            fill(OPU, op_unit, "b1a", 0, n, 256)
        for n in range(2):
            fill(OPU, op_unit, "b1b", 0, n, 384)

        assert not sc_ready
        assert not mask_pending

    nc.finalize()
    return nc


def _get_nc():
    with _lock:
        if "nc" not in _cache:
            _cache["nc"] = _build_nc()
        return _cache["nc"]


def _shard_inputs(x, input_ids, Wq, bq, Wk, bk, Wv, bv, Wo, bo):
    import ml_dtypes
    bf16 = ml_dtypes.bfloat16

    x = np.asarray(x, dtype=np.float32)
    xT = np.ascontiguousarray(x.reshape(R, D).T).astype(bf16)
    woT = np.ascontiguousarray(np.asarray(Wo, dtype=np.float32).T).astype(bf16)
    bo_f = np.asarray(bo, dtype=np.float32)
    ids = np.asarray(input_ids).astype(np.int32)
    # ids_r[p, b*NKT + t] = input_ids[b, t*128 + p]
    ids_r = np.ascontiguousarray(ids.reshape(B, NKT, 128).transpose(2, 0, 1)
                                 .reshape(128, B * NKT))
    Wq = np.asarray(Wq, dtype=np.float32)
    Wk = np.asarray(Wk, dtype=np.float32)
    Wv = np.asarray(Wv, dtype=np.float32)
    bq = np.asarray(bq, dtype=np.float32)
    bk = np.asarray(bk, dtype=np.float32)
    bv = np.asarray(bv, dtype=np.float32)

    in_maps = []
    for c in range(NCORES):
        sl = slice(c * LD, (c + 1) * LD)
        in_maps.append({
            "xT": xT,
            "wqT": np.ascontiguousarray(Wq[sl].T).astype(bf16),
            "wkT": np.ascontiguousarray(Wk[sl].T).astype(bf16),
            "wvT": np.ascontiguousarray(Wv[sl].T).astype(bf16),
            "woT": woT,
            "bq": bq[sl].copy(),
            "bk": bk[sl].copy(),
            "bv": bv[sl].copy(),
            "bo": bo_f,
            "ids": ids_r,
        })
    return in_maps


def run(trace=False, **inputs):
    """Run the kernel; returns (output, BassKernelResults)."""
    from concourse.bass_utils import run_bass_kernel_spmd

    nc = _get_nc()
    in_maps = _shard_inputs(**inputs)
    res = run_bass_kernel_spmd(nc, in_maps, core_ids=list(range(NCORES)),
                               trace=trace)
    full = np.empty((B, S, D), dtype=np.float32)
    for j in range(NCORES):
        o = np.asarray(res.results[j]["out"], dtype=np.float32)
        # batch 0: one A2A, core j owns 256 contiguous queries
        full[0, 256 * j:256 * (j + 1), :] = o[0:256, :]
        # batch 1: two A2A halves, 128 queries per core each
        for h2 in range(2):
            full[1, 1024 * h2 + 128 * j:1024 * h2 + 128 * (j + 1), :] = \
                o[256 + h2 * 128:256 + (h2 + 1) * 128, :]
    return full, res


def kernel(**inputs) -> np.ndarray:
    full, _ = run(trace=False, **inputs)
    return full
